# revision 9
# baseline (speedup 1.0000x reference)
"""MixHopNet GNN kernel for 8 Trainium2 NeuronCores (Bass/Tile SPMD).

Math (reference): GCN-normalized adjacency A = D^-1/2 (Adj + I) D^-1/2 over
N=50000 nodes / 800k random edges (+self loops), then
  x1 = A x ; x2 = A x1
  h  = relu([x w1_0 + b1_0, x1 w1_1 + b1_1, x2 w1_2 + b1_2])
  out = log_softmax([h w2_0 + b2_0, (A h) w2_1 + b2_1])

Distribution (graph/data parallel): nodes are packed into 456 blocks of 128
slots (degree-balanced bin packing), 57 blocks per core.  Propagation for a
dst block accumulates S_j^T @ V_j over edge chunks j of 128 edges in PSUM,
where V_j is a dma_gather of the bf16 source-row table and S_j is a PURE 0/1
one-hot (edge -> dst slot) PRECOMPUTED ON HOST and streamed from HBM as bf16
(on-HW DVE tensor_scalar one-hot builds cost ~1.7us each; streaming is ~free).
The two D^-1/2 factors: gather tables are pre-scaled by dinv[src], and
dinv[dst] is applied once per block at PSUM eviction (it factors out of the
edge sum).  Reference self-loops are NOT in the edge list; they are the
identity matmul of the block's own (locally available) table rows.

dma_gather descriptor generation is the critical resource (~7ns/descriptor,
measured); calls are spread round-robin over all 4 SWDGE queues and use
exact per-(block,half) chunk counts padded to the max over cores at the same
sorted block position (so the SPMD program is identical on every core).

Cross-core: tables are replicated; x1 and z1 = dinv*(h @ w2_1) shards are
AllGather'd between propagations ((A h) w2_1 == A (h w2_1), so only 40 cols
propagate in conv2).  dma_gather needs int16 indices, so tables are split in
two 29184-row halves aligned with the node-id split.
"""
import sys

sys.path.insert(0, "/opt/trn_rl_repo")

import numpy as np

import concourse.bass as bass  # noqa: F401
import concourse.bacc as bacc
import concourse.tile as tile
from concourse import mybir
from concourse.bass_utils import run_bass_kernel_spmd

import ml_dtypes

BF16 = ml_dtypes.bfloat16

# ---- problem constants (hardcoded; kernel.py must be self-contained) ----
N = 50000
FIN = 128
H = 128
CO = 40
NCORES = 8
P = 128
NB = 456               # node blocks total
BPC = NB // NCORES     # 57 blocks per core
S = NB * P             # 58368 slots
NSH = BPC * P          # 7296 slots per core
HALF = S // 2          # 29184  (int16-addressable table half)
NHALF = N // 2         # node-id split matching the slot-half split
NQ = 4                 # SWDGE queues

LAST_RESULT = None     # BassKernelResults of the most recent run (for test.py)

_COMPILED = {}


# --------------------------------------------------------------------------
# host-side preprocessing
# --------------------------------------------------------------------------
def _pack_nodes(a, b):
    """Assign each node a slot: nodes [0,NHALF) -> blocks [0,NB/2), rest ->
    blocks [NB/2,NB).  Greedy 2D bin packing (descending total degree,
    minimize max(lo_load, hi_load)) balances both src-half in-edge counts."""
    node2slot = np.empty(N, np.int64)
    for hstart, hend, b0 in ((0, NHALF, 0), (NHALF, N, NB // 2)):
        nbins = NB // 2
        nodes = np.arange(hstart, hend)
        nodes = nodes[np.argsort(-(a[nodes] + b[nodes]), kind="stable")]
        lo_load = np.zeros(nbins, np.int64)
        hi_load = np.zeros(nbins, np.int64)
        cnt = np.zeros(nbins, np.int64)
        av = a[nodes]
        bv = b[nodes]
        for i in range(nodes.shape[0]):
            score = np.maximum(lo_load + av[i], hi_load + bv[i])
            score[cnt >= P] = 1 << 60
            blk = int(np.argmin(score))
            node2slot[nodes[i]] = (b0 + blk) * P + cnt[blk]
            cnt[blk] += 1
            lo_load[blk] += av[i]
            hi_load[blk] += bv[i]
    return node2slot


def _wrap_idx(flat):
    """[n*128] int16 -> [128, n*8] (16-partition wrap, replicated 8x)."""
    n = flat.shape[0] // 128
    arr = flat.reshape(n * 8, 16).T.copy()
    return np.tile(arr, (8, 1))


def _preprocess(x, edge_index, w1_0, b1_0, w1_1, b1_1, w1_2, b1_2,
                w2_0, b2_0, w2_1, b2_1):
    src = edge_index[0].astype(np.int64)
    dst = edge_index[1].astype(np.int64)
    E = src.shape[0]

    deg = (np.bincount(dst, minlength=N) + 1).astype(np.float32)  # +self loop
    dinv = (1.0 / np.sqrt(deg)).astype(np.float32)

    islo_n = src < NHALF
    a = np.bincount(dst[islo_n], minlength=N)
    b = np.bincount(dst[~islo_n], minlength=N)
    node2slot = _pack_nodes(a, b)

    # per-(old block, src half) edge counts, to sort/deal blocks to cores
    blk_old = node2slot[dst] >> 7
    grp_old = blk_old * 2 + (~islo_n).astype(np.int64)
    cnts = np.bincount(grp_old, minlength=2 * NB).reshape(NB, 2)
    tot = cnts.sum(axis=1)
    # block -> position: snake-deal blocks (sorted by total edges desc)
    # within each half so every core gets a similar block-size profile.
    order_bs = np.empty(NB, np.int64)  # new position -> old block id
    blk_perm = np.empty(NB, np.int64)  # old block id -> new position
    for half, coff in ((0, 0), (1, 4)):
        ids = np.arange(half * (NB // 2), (half + 1) * (NB // 2))
        ids = ids[np.argsort(-tot[ids], kind="stable")]
        percore = [[] for _ in range(4)]
        for i, bid in enumerate(ids):
            c = i % 4 if (i // 4) % 2 == 0 else 3 - i % 4
            percore[c].append(bid)
        for c in range(4):
            for j, bid in enumerate(percore[c]):
                pos = (coff + c) * BPC + j
                order_bs[pos] = bid
                blk_perm[bid] = pos
    slot_perm = (blk_perm[:, None] * P + np.arange(P)[None, :]).reshape(-1)
    node2slot = slot_perm[node2slot]
    cnts = cnts[order_bs]              # [position, half] edge counts

    dslot = node2slot[dst]
    nblk = dslot >> 7                  # block position (0..NB-1)
    d_in_blk = dslot & 127
    sslot = node2slot[src]
    islo = sslot < HALF                # == islo_n (halves preserved)
    sidx = np.where(islo, sslot, sslot - HALF).astype(np.int16)

    # chunk counts per (within-core position, half): max across cores
    ch = np.ceil(cnts / P).astype(np.int64).reshape(NCORES, BPC, 2)
    K_lo = np.maximum(ch[:, :, 0].max(axis=0), 1)  # [BPC]
    K_hi = np.maximum(ch[:, :, 1].max(axis=0), 1)
    Ktot = K_lo + K_hi
    TOTCH = int(Ktot.sum())            # chunks per core

    # flatten edges into the padded chunk layout
    grp = nblk * 2 + (~islo).astype(np.int64)
    order = np.argsort(grp, kind="stable")
    gs = np.bincount(grp, minlength=2 * NB)
    starts = np.concatenate([[0], np.cumsum(gs)[:-1]])
    epos = np.arange(E) - starts[grp[order]]    # position within group

    base_lo = np.concatenate([[0], np.cumsum(Ktot)[:-1]])  # [BPC] chunk base
    base_hi = base_lo + K_lo
    posc = np.arange(NB) % BPC
    gbase = np.stack([base_lo[posc], base_hi[posc]], axis=1)  # [NB, 2]
    eslot = gbase[nblk[order], (~islo[order]).astype(np.int64)] * P + epos
    core_e = nblk[order] // BPC

    sidx_pad = np.zeros((NCORES, TOTCH * P), np.int16)
    sidx_pad[core_e, eslot] = sidx[order]
    # per-edge dst-slot compare values for the on-DVE one-hot build;
    # padding entries get 999 (matches no iota lane -> all-zero column)
    ldst_all = np.full((NCORES, TOTCH * P), 999.0, np.float32)
    ldst_all[core_e, eslot] = d_in_blk[order].astype(np.float32)

    dinv_slot = np.zeros(S, np.float32)
    dinv_slot[node2slot] = dinv
    x_slot = np.zeros((S, FIN), np.float32)
    x_slot[node2slot] = x
    u0 = (x_slot * dinv_slot[:, None]).astype(BF16)

    brow = np.tile(np.concatenate([b2_0, b2_1])[None, :], (P, 1)).astype(np.float32)
    w1s = np.concatenate([w1_0, w1_1, w1_2], axis=1).astype(BF16)
    b1m = np.stack([b1_0, b1_1, b1_2], axis=1).astype(np.float32)
    ident = np.eye(P, dtype=BF16)

    in_maps = []
    for c in range(NCORES):
        rows = slice(c * NSH, (c + 1) * NSH)
        dm = dinv_slot[rows].reshape(BPC, P).T.copy()
        in_maps.append(dict(
            u0=u0,
            u0own=u0[rows],
            xT=np.ascontiguousarray(x_slot[rows].T).astype(BF16),
            idx=_wrap_idx(sidx_pad[c]),
            ldst=np.ascontiguousarray(ldst_all[c].reshape(TOTCH, P).T),
            iota=np.tile(np.arange(P, dtype=np.float32), (P, 1)).astype(BF16),
            dinvc=dm,
            dinv2c=(dm * dm),
            ident=ident,
            w1s=w1s,
            b1m=b1m,
            w2a=np.asarray(w2_0, np.float32).astype(BF16),
            w2b=np.asarray(w2_1, np.float32).astype(BF16),
            brow=brow,
        ))
    return in_maps, node2slot, tuple(int(v) for v in K_lo), \
        tuple(int(v) for v in K_hi)


# --------------------------------------------------------------------------
# device program
# --------------------------------------------------------------------------
def _build(nc, K_lo, K_hi, stage="full"):
    dt = mybir.dt
    f32 = dt.float32
    bf16 = dt.bfloat16
    Ktot = [a + b for a, b in zip(K_lo, K_hi)]
    TOTCH = sum(Ktot)
    KLO_M, KHI_M, KT_M = max(K_lo), max(K_hi), max(Ktot)
    cbase = [0]
    for k in Ktot:
        cbase.append(cbase[-1] + k)

    u0 = nc.dram_tensor("u0", [S, FIN], bf16, kind="ExternalInput").ap()
    u0own = nc.dram_tensor("u0own", [NSH, FIN], bf16, kind="ExternalInput").ap()
    xT = nc.dram_tensor("xT", [P, NSH], bf16, kind="ExternalInput").ap()
    idx = nc.dram_tensor("idx", [P, TOTCH * 8], dt.int16, kind="ExternalInput").ap()
    ldst = nc.dram_tensor("ldst", [P, TOTCH], f32, kind="ExternalInput").ap()
    iota = nc.dram_tensor("iota", [P, P], bf16, kind="ExternalInput").ap()
    dinvc = nc.dram_tensor("dinvc", [P, BPC], f32, kind="ExternalInput").ap()
    dinv2c = nc.dram_tensor("dinv2c", [P, BPC], f32, kind="ExternalInput").ap()
    identd = nc.dram_tensor("ident", [P, P], bf16, kind="ExternalInput").ap()
    w1s = nc.dram_tensor("w1s", [P, 3 * H], bf16, kind="ExternalInput").ap()
    b1m = nc.dram_tensor("b1m", [P, 3], f32, kind="ExternalInput").ap()
    w2a = nc.dram_tensor("w2a", [3 * H, CO], bf16, kind="ExternalInput").ap()
    w2b = nc.dram_tensor("w2b", [3 * H, CO], bf16, kind="ExternalInput").ap()
    brow = nc.dram_tensor("brow", [P, 2 * CO], f32, kind="ExternalInput").ap()
    out = nc.dram_tensor("out", [NSH, 2 * CO], f32, kind="ExternalOutput").ap()

    rg = [list(range(NCORES))]

    with tile.TileContext(nc) as tc:
        with (
            tc.tile_pool(name="res", bufs=1) as res,
            tc.tile_pool(name="dram", bufs=1, space="DRAM") as dram,
        ):
            def load(name, src_ap, shape, dtype=f32):
                t = res.tile(shape, dtype, tag=name, name=name)
                nc.sync.dma_start(out=t[:], in_=src_ap)
                return t

            idx_t = load("idx", idx[:], [P, TOTCH * 8], dt.int16)
            ldst_t = load("ldst", ldst[:], [P, TOTCH])
            iota_t = load("iota", iota[:], [P, P], bf16)
            dinvc_t = load("dinvc", dinvc[:], [P, BPC])
            dinv2c_t = load("dinv2c", dinv2c[:], [P, BPC])
            ident_t = load("ident", identd[:], [P, P], bf16)
            w1_t = load("w1s", w1s[:], [P, 3 * H], bf16)
            b1_t = load("b1m", b1m[:], [P, 3])
            brow_t = load("brow", brow[:], [P, 2 * CO])
            w2a_t = [load(f"w2a{i}", w2a[i * H:(i + 1) * H, :], [P, CO], bf16)
                     for i in range(3)]
            w2b_t = [load(f"w2b{i}", w2b[i * H:(i + 1) * H, :], [P, CO], bf16)
                     for i in range(3)]

            x1T = res.tile([P, NSH], bf16, tag="x1T")
            x2T = res.tile([P, NSH], bf16, tag="x2T")
            hT = [res.tile([P, NSH], bf16, tag=f"hT{i}", name=f"hT{i}")
                  for i in range(3)]
            out80 = res.tile([P, BPC * 2 * CO], f32, tag="out80")

            u1b = dram.tile([NSH, FIN], bf16, tag="u1b")
            u1f = dram.tile([S, FIN], bf16, tag="u1f", addr_space="Shared")
            uzb = dram.tile([NSH, P], bf16, tag="uzb")
            uzf = dram.tile([S, P], bf16, tag="uzf", addr_space="Shared")

            qn = [0]  # SWDGE queue round-robin

            def gather_half(pw, tag, tbl, b, Ks, base_off, KM):
                Kp = Ks[b]
                v = pw.tile([P, KM, FIN], bf16, tag=tag, name=tag)
                o = base_off[b]
                nc.gpsimd.dma_gather(
                    v[:, 0:Kp, :], tbl, idx_t[:, o * 8:(o + Kp) * 8],
                    num_idxs=Kp * P, num_idxs_reg=Kp * P, elem_size=FIN,
                    queue_num=qn[0])
                qn[0] = (qn[0] + 1) % NQ
                return v

            base_lo = [cbase[b] for b in range(BPC)]
            base_hi = [cbase[b] + K_lo[b] for b in range(BPC)]

            def prop(tbl, own, width, evict, pools):
                pw, pp, ohp, sp = pools
                for b in range(BPC):
                    vlo = gather_half(pw, "vlo", tbl[0:HALF, :], b, K_lo,
                                      base_lo, KLO_M)
                    vhi = gather_half(pw, "vhi", tbl[HALF:S, :], b, K_hi,
                                      base_hi, KHI_M)
                    oht = ohp.tile([P, KT_M * P], bf16, tag="oht")
                    for j in range(Ktot[b]):
                        nc.vector.tensor_scalar(
                            out=oht[:, j * P:(j + 1) * P], in0=iota_t[:],
                            scalar1=ldst_t[:, cbase[b] + j:cbase[b] + j + 1],
                            scalar2=None, op0=mybir.AluOpType.is_equal)
                    sblk = sp.tile([P, FIN], bf16, tag="sblk")
                    nc.sync.dma_start(out=sblk[:],
                                      in_=own[b * P:(b + 1) * P, :])
                    ps = pp.tile([P, width], f32, tag="agg")
                    nc.tensor.matmul(out=ps[:], lhsT=ident_t[:],
                                     rhs=sblk[:, 0:width],
                                     start=True, stop=False)
                    for j in range(Ktot[b]):
                        srcv = (vlo[:, j, 0:width] if j < K_lo[b]
                                else vhi[:, j - K_lo[b], 0:width])
                        nc.tensor.matmul(
                            out=ps[:],
                            lhsT=oht[:, j * P:(j + 1) * P],
                            rhs=srcv,
                            start=False, stop=(j == Ktot[b] - 1))
                    evict(b, ps)

            # ================= P1: x1 = A x =================
            with (
                tc.tile_pool(name="p1w", bufs=3) as pw,
                tc.tile_pool(name="p1p", bufs=4, space="PSUM") as pp,
                tc.tile_pool(name="p1o", bufs=2) as ohp,
                tc.tile_pool(name="p1s", bufs=2) as sp,
                tc.tile_pool(name="p1e", bufs=3) as evp,
                tc.tile_pool(name="p1t", bufs=2, space="PSUM") as tpp,
            ):
                def evict1(b, ps):
                    x1t = evp.tile([P, P], bf16, tag="x1t")
                    nc.scalar.mul(x1t[:], ps[:], dinvc_t[:, b:b + 1])
                    u1t = evp.tile([P, P], bf16, tag="u1t")
                    nc.scalar.mul(u1t[:], ps[:], dinv2c_t[:, b:b + 1])
                    nc.sync.dma_start(out=u1b[b * P:(b + 1) * P, :], in_=u1t[:])
                    trp = tpp.tile([P, P], bf16, tag="trp")
                    nc.tensor.transpose(out=trp[:], in_=x1t[:],
                                        identity=ident_t[:])
                    nc.vector.tensor_copy(out=x1T[:, b * P:(b + 1) * P],
                                          in_=trp[:])

                prop(u0, u0own, FIN, evict1, (pw, pp, ohp, sp))

            nc.gpsimd.collective_compute(
                "AllGather", mybir.AluOpType.bypass, replica_groups=rg,
                ins=[u1b.opt()], outs=[u1f.opt()])

            if stage == "p1":
                dbg = nc.dram_tensor("dbg", [S, FIN], f32,
                                     kind="ExternalOutput").ap()
                with tc.tile_pool(name="dbgp", bufs=2) as dp:
                    for r0 in range(0, S, P):
                        t = dp.tile([P, FIN], f32, tag="dbgt")
                        nc.vector.tensor_copy(out=t[:], in_=u1f[r0:r0 + P, :])
                        nc.sync.dma_start(out=dbg[r0:r0 + P, :], in_=t[:])
                return

            # ================= P2: x2 = A x1 =================
            with (
                tc.tile_pool(name="p2w", bufs=3) as pw,
                tc.tile_pool(name="p2p", bufs=4, space="PSUM") as pp,
                tc.tile_pool(name="p2o", bufs=2) as ohp,
                tc.tile_pool(name="p2s", bufs=2) as sp,
                tc.tile_pool(name="p2e", bufs=3) as evp,
                tc.tile_pool(name="p2t", bufs=2, space="PSUM") as tpp,
            ):
                def evict2(b, ps):
                    x2t = evp.tile([P, P], bf16, tag="x2t")
                    nc.scalar.mul(x2t[:], ps[:], dinvc_t[:, b:b + 1])
                    trp = tpp.tile([P, P], bf16, tag="trp2")
                    nc.tensor.transpose(out=trp[:], in_=x2t[:],
                                        identity=ident_t[:])
                    nc.vector.tensor_copy(out=x2T[:, b * P:(b + 1) * P],
                                          in_=trp[:])

                prop(u1f, u1b, FIN, evict2, (pw, pp, ohp, sp))

            if stage == "p2":
                dbg = nc.dram_tensor("dbg", [2 * P, NSH], f32,
                                     kind="ExternalOutput").ap()
                t = res.tile([P, NSH], f32, tag="dbgt")
                nc.vector.tensor_copy(out=t[:], in_=x1T[:])
                nc.sync.dma_start(out=dbg[0:P, :], in_=t[:])
                nc.vector.tensor_copy(out=t[:], in_=x2T[:])
                nc.sync.dma_start(out=dbg[P:2 * P, :], in_=t[:])
                return

            # ================= dense: hT = relu(w1^T [x|x1|x2]^T + b1) ======
            with (
                tc.tile_pool(name="dxs", bufs=3) as xsp,
                tc.tile_pool(name="dps", bufs=3, space="PSUM") as hpp,
            ):
                for f0 in range(0, NSH, 512):
                    w = min(512, NSH - f0)
                    xt = xsp.tile([P, 512], bf16, tag="xs")
                    nc.sync.dma_start(out=xt[:, 0:w], in_=xT[:, f0:f0 + w])
                    srcs = (xt[:, 0:w], x1T[:, f0:f0 + w], x2T[:, f0:f0 + w])
                    for i in range(3):
                        ph = hpp.tile([P, 512], f32, tag="hps")
                        nc.tensor.matmul(out=ph[:, 0:w],
                                         lhsT=w1_t[:, i * H:(i + 1) * H],
                                         rhs=srcs[i], start=True, stop=True)
                        nc.scalar.activation(
                            out=hT[i][:, f0:f0 + w], in_=ph[:, 0:w],
                            func=mybir.ActivationFunctionType.Relu,
                            bias=b1_t[:, i:i + 1], scale=1.0)

            if stage == "dense":
                dbg = nc.dram_tensor("dbg", [3 * P, NSH], f32,
                                     kind="ExternalOutput").ap()
                t = res.tile([P, NSH], f32, tag="dbgt")
                for i in range(3):
                    nc.vector.tensor_copy(out=t[:], in_=hT[i][:])
                    nc.sync.dma_start(out=dbg[i * P:(i + 1) * P, :], in_=t[:])
                return

            # ================= z1 = dinv*(h w2_1) (-> uz), out1 = h w2_0 ====
            with (
                tc.tile_pool(name="eps", bufs=4, space="PSUM") as zpp,
                tc.tile_pool(name="eev", bufs=3) as evp,
            ):
                for b in range(BPC):
                    pz = zpp.tile([P, CO], f32, tag="pz")
                    for i in range(3):
                        nc.tensor.matmul(out=pz[:],
                                         lhsT=hT[i][:, b * P:(b + 1) * P],
                                         rhs=w2b_t[i][:], start=(i == 0),
                                         stop=(i == 2))
                    uzt = evp.tile([P, P], bf16, tag="uzt")
                    nc.vector.memset(uzt[:, CO:P], 0)
                    nc.scalar.mul(uzt[:, 0:CO], pz[:], dinvc_t[:, b:b + 1])
                    nc.sync.dma_start(out=uzb[b * P:(b + 1) * P, :], in_=uzt[:])
                    po = zpp.tile([P, CO], f32, tag="po")
                    for i in range(3):
                        nc.tensor.matmul(out=po[:],
                                         lhsT=hT[i][:, b * P:(b + 1) * P],
                                         rhs=w2a_t[i][:], start=(i == 0),
                                         stop=(i == 2))
                    nc.vector.tensor_copy(
                        out=out80[:, b * 2 * CO:b * 2 * CO + CO], in_=po[:])

            nc.gpsimd.collective_compute(
                "AllGather", mybir.AluOpType.bypass, replica_groups=rg,
                ins=[uzb.opt()], outs=[uzf.opt()])

            # ========== P3: out2 = dinv * A' z1, fused bias+softmax-head ====
            # Per-block softmax prologue (bias, max, shift, exp+accum) rides
            # inside the gather-bound P3 loop; the Ln over all 57 row-sums is
            # ONE activation at the end (the ACT engine reloads its function
            # table on Exp<->Ln switches, 1.3us each -- batching avoids 2*57
            # reloads).
            sh_all = res.tile([P, BPC * 2 * CO], f32, tag="sh_all")
            se_all = res.tile([P, BPC], f32, tag="se_all")
            LAG = 6  # softmax prologue trails the prop by LAG blocks so the
            #          exp on ACT never head-of-line-blocks the PSUM-releasing
            #          mul of the current block.
            with (
                tc.tile_pool(name="p3w", bufs=3) as pw,
                tc.tile_pool(name="p3p", bufs=4, space="PSUM") as pp,
                tc.tile_pool(name="p3o", bufs=2) as ohp,
                tc.tile_pool(name="p3s", bufs=2) as sp,
                tc.tile_pool(name="p3f", bufs=3) as fp,
            ):
                def smax_head(b):
                    t1 = fp.tile([P, 2 * CO], f32, tag="f1")
                    nc.vector.tensor_tensor(
                        out=t1[:], in0=out80[:, b * 2 * CO:(b + 1) * 2 * CO],
                        in1=brow_t[:], op=mybir.AluOpType.add)
                    mx = fp.tile([P, 1], f32, tag="mx")
                    nc.vector.reduce_max(out=mx[:], in_=t1[:],
                                         axis=mybir.AxisListType.X)
                    sh = sh_all[:, b * 2 * CO:(b + 1) * 2 * CO]
                    nc.vector.tensor_scalar(out=sh, in0=t1[:], scalar1=mx[:],
                                            scalar2=None,
                                            op0=mybir.AluOpType.subtract)
                    ex = fp.tile([P, 2 * CO], f32, tag="ex")
                    nc.scalar.activation(out=ex[:], in_=sh,
                                         func=mybir.ActivationFunctionType.Exp,
                                         accum_out=se_all[:, b:b + 1])

                def evict3(b, ps):
                    nc.scalar.mul(
                        out80[:, b * 2 * CO + CO:(b + 1) * 2 * CO], ps[:],
                        dinvc_t[:, b:b + 1])
                    if b >= LAG:
                        smax_head(b - LAG)

                prop(uzf, uzb, CO, evict3, (pw, pp, ohp, sp))
                for b in range(BPC - LAG, BPC):
                    smax_head(b)

            # ================= lse + final subtract + store ==================
            with tc.tile_pool(name="fin", bufs=3) as fp:
                lse = res.tile([P, BPC], f32, tag="lse")
                nc.scalar.activation(out=lse[:], in_=se_all[:],
                                     func=mybir.ActivationFunctionType.Ln)
                for b in range(BPC):
                    r = fp.tile([P, 2 * CO], f32, tag="r")
                    nc.vector.tensor_scalar(
                        out=r[:], in0=sh_all[:, b * 2 * CO:(b + 1) * 2 * CO],
                        scalar1=lse[:, b:b + 1], scalar2=None,
                        op0=mybir.AluOpType.subtract)
                    nc.sync.dma_start(out=out[b * P:(b + 1) * P, :], in_=r[:])


def _get_compiled(K_lo, K_hi, stage="full"):
    key = (K_lo, K_hi, stage)
    if key not in _COMPILED:
        nc = bacc.Bacc("TRN2", target_bir_lowering=False, debug=False,
                       num_devices=NCORES, num_swdge_queues=NQ)
        _build(nc, K_lo, K_hi, stage=stage)
        nc.compile()
        _COMPILED[key] = nc
    return _COMPILED[key]


def kernel(**inputs):
    global LAST_RESULT
    args = {k: np.asarray(v) for k, v in inputs.items()}
    in_maps, node2slot, K_lo, K_hi = _preprocess(
        args["x"].astype(np.float32), args["edge_index"],
        args["w1_0"].astype(np.float32), args["b1_0"].astype(np.float32),
        args["w1_1"].astype(np.float32), args["b1_1"].astype(np.float32),
        args["w1_2"].astype(np.float32), args["b1_2"].astype(np.float32),
        args["w2_0"].astype(np.float32), args["b2_0"].astype(np.float32),
        args["w2_1"].astype(np.float32), args["b2_1"].astype(np.float32),
    )
    nc = _get_compiled(K_lo, K_hi)
    res = run_bass_kernel_spmd(nc, in_maps, list(range(NCORES)))
    LAST_RESULT = res
    out_slot = np.concatenate([res.results[c]["out"] for c in range(NCORES)],
                              axis=0)
    return out_slot[node2slot].astype(np.float32)


# revision 10
# speedup vs baseline: 1.0099x; 1.0099x over previous
"""MixHopNet GNN kernel for 8 Trainium2 NeuronCores (Bass/Tile SPMD).

Math (reference): GCN-normalized adjacency A = D^-1/2 (Adj + I) D^-1/2 over
N=50000 nodes / 800k random edges (+self loops), then
  x1 = A x ; x2 = A x1
  h  = relu([x w1_0 + b1_0, x1 w1_1 + b1_1, x2 w1_2 + b1_2])
  out = log_softmax([h w2_0 + b2_0, (A h) w2_1 + b2_1])

Distribution (graph/data parallel): nodes are packed into 456 blocks of 128
slots (degree-balanced bin packing), 57 blocks per core.  Propagation for a
dst block accumulates S_j^T @ V_j over edge chunks j of 128 edges in PSUM,
where V_j is a dma_gather of the bf16 source-row table and S_j is a PURE 0/1
one-hot (edge -> dst slot) PRECOMPUTED ON HOST and streamed from HBM as bf16
(on-HW DVE tensor_scalar one-hot builds cost ~1.7us each; streaming is ~free).
The two D^-1/2 factors: gather tables are pre-scaled by dinv[src], and
dinv[dst] is applied once per block at PSUM eviction (it factors out of the
edge sum).  Reference self-loops are NOT in the edge list; they are the
identity matmul of the block's own (locally available) table rows.

dma_gather descriptor generation is the critical resource (~7ns/descriptor,
measured); calls are spread round-robin over all 4 SWDGE queues and use
exact per-(block,half) chunk counts padded to the max over cores at the same
sorted block position (so the SPMD program is identical on every core).

Cross-core: tables are replicated; x1 and z1 = dinv*(h @ w2_1) shards are
AllGather'd between propagations ((A h) w2_1 == A (h w2_1), so only 40 cols
propagate in conv2).  dma_gather needs int16 indices, so tables are split in
two 29184-row halves aligned with the node-id split.
"""
import sys

sys.path.insert(0, "/opt/trn_rl_repo")

import numpy as np

import concourse.bass as bass  # noqa: F401
import concourse.bacc as bacc
import concourse.tile as tile
from concourse import mybir
from concourse.bass_utils import run_bass_kernel_spmd

import ml_dtypes

BF16 = ml_dtypes.bfloat16

# ---- problem constants (hardcoded; kernel.py must be self-contained) ----
N = 50000
FIN = 128
H = 128
CO = 40
NCORES = 8
P = 128
NB = 456               # node blocks total
BPC = NB // NCORES     # 57 blocks per core
S = NB * P             # 58368 slots
NSH = BPC * P          # 7296 slots per core
HALF = S // 2          # 29184  (int16-addressable table half)
NHALF = N // 2         # node-id split matching the slot-half split
NQ = 4                 # SWDGE queues

LAST_RESULT = None     # BassKernelResults of the most recent run (for test.py)

_COMPILED = {}


# --------------------------------------------------------------------------
# host-side preprocessing
# --------------------------------------------------------------------------
def _pack_nodes(a, b):
    """Assign each node a slot: nodes [0,NHALF) -> blocks [0,NB/2), rest ->
    blocks [NB/2,NB).  Greedy 2D bin packing (descending total degree,
    minimize max(lo_load, hi_load)) balances both src-half in-edge counts."""
    node2slot = np.empty(N, np.int64)
    for hstart, hend, b0 in ((0, NHALF, 0), (NHALF, N, NB // 2)):
        nbins = NB // 2
        nodes = np.arange(hstart, hend)
        nodes = nodes[np.argsort(-(a[nodes] + b[nodes]), kind="stable")]
        lo_load = np.zeros(nbins, np.int64)
        hi_load = np.zeros(nbins, np.int64)
        cnt = np.zeros(nbins, np.int64)
        av = a[nodes]
        bv = b[nodes]
        for i in range(nodes.shape[0]):
            score = np.maximum(lo_load + av[i], hi_load + bv[i])
            score[cnt >= P] = 1 << 60
            blk = int(np.argmin(score))
            node2slot[nodes[i]] = (b0 + blk) * P + cnt[blk]
            cnt[blk] += 1
            lo_load[blk] += av[i]
            hi_load[blk] += bv[i]
    return node2slot


def _wrap_idx(flat):
    """[n*128] int16 -> [128, n*8] (16-partition wrap, replicated 8x)."""
    n = flat.shape[0] // 128
    arr = flat.reshape(n * 8, 16).T.copy()
    return np.tile(arr, (8, 1))


def _preprocess(x, edge_index, w1_0, b1_0, w1_1, b1_1, w1_2, b1_2,
                w2_0, b2_0, w2_1, b2_1):
    src = edge_index[0].astype(np.int64)
    dst = edge_index[1].astype(np.int64)
    E = src.shape[0]

    deg = (np.bincount(dst, minlength=N) + 1).astype(np.float32)  # +self loop
    dinv = (1.0 / np.sqrt(deg)).astype(np.float32)

    islo_n = src < NHALF
    a = np.bincount(dst[islo_n], minlength=N)
    b = np.bincount(dst[~islo_n], minlength=N)
    node2slot = _pack_nodes(a, b)

    # per-(old block, src half) edge counts, to sort/deal blocks to cores
    blk_old = node2slot[dst] >> 7
    grp_old = blk_old * 2 + (~islo_n).astype(np.int64)
    cnts = np.bincount(grp_old, minlength=2 * NB).reshape(NB, 2)
    tot = cnts.sum(axis=1)
    # block -> position: snake-deal blocks (sorted by total edges desc)
    # within each half so every core gets a similar block-size profile.
    order_bs = np.empty(NB, np.int64)  # new position -> old block id
    blk_perm = np.empty(NB, np.int64)  # old block id -> new position
    for half, coff in ((0, 0), (1, 4)):
        ids = np.arange(half * (NB // 2), (half + 1) * (NB // 2))
        ids = ids[np.argsort(-tot[ids], kind="stable")]
        percore = [[] for _ in range(4)]
        for i, bid in enumerate(ids):
            c = i % 4 if (i // 4) % 2 == 0 else 3 - i % 4
            percore[c].append(bid)
        for c in range(4):
            for j, bid in enumerate(percore[c]):
                pos = (coff + c) * BPC + j
                order_bs[pos] = bid
                blk_perm[bid] = pos
    slot_perm = (blk_perm[:, None] * P + np.arange(P)[None, :]).reshape(-1)
    node2slot = slot_perm[node2slot]
    cnts = cnts[order_bs]              # [position, half] edge counts

    dslot = node2slot[dst]
    nblk = dslot >> 7                  # block position (0..NB-1)
    d_in_blk = dslot & 127
    sslot = node2slot[src]
    islo = sslot < HALF                # == islo_n (halves preserved)
    sidx = np.where(islo, sslot, sslot - HALF).astype(np.int16)

    # chunk counts per (within-core position, half): max across cores
    ch = np.ceil(cnts / P).astype(np.int64).reshape(NCORES, BPC, 2)
    K_lo = np.maximum(ch[:, :, 0].max(axis=0), 1)  # [BPC]
    K_hi = np.maximum(ch[:, :, 1].max(axis=0), 1)
    Ktot = K_lo + K_hi
    TOTCH = int(Ktot.sum())            # chunks per core

    # flatten edges into the padded chunk layout
    grp = nblk * 2 + (~islo).astype(np.int64)
    order = np.argsort(grp, kind="stable")
    gs = np.bincount(grp, minlength=2 * NB)
    starts = np.concatenate([[0], np.cumsum(gs)[:-1]])
    epos = np.arange(E) - starts[grp[order]]    # position within group

    base_lo = np.concatenate([[0], np.cumsum(Ktot)[:-1]])  # [BPC] chunk base
    base_hi = base_lo + K_lo
    posc = np.arange(NB) % BPC
    gbase = np.stack([base_lo[posc], base_hi[posc]], axis=1)  # [NB, 2]
    eslot = gbase[nblk[order], (~islo[order]).astype(np.int64)] * P + epos
    core_e = nblk[order] // BPC

    sidx_pad = np.zeros((NCORES, TOTCH * P), np.int16)
    sidx_pad[core_e, eslot] = sidx[order]
    # per-edge dst-slot compare values for the on-DVE one-hot build;
    # padding entries get 999 (matches no iota lane -> all-zero column)
    ldst_all = np.full((NCORES, TOTCH * P), 999.0, np.float32)
    ldst_all[core_e, eslot] = d_in_blk[order].astype(np.float32)

    dinv_slot = np.zeros(S, np.float32)
    dinv_slot[node2slot] = dinv
    x_slot = np.zeros((S, FIN), np.float32)
    x_slot[node2slot] = x
    u0 = (x_slot * dinv_slot[:, None]).astype(BF16)

    brow = np.tile(np.concatenate([b2_0, b2_1])[None, :], (P, 1)).astype(np.float32)
    w1s = np.concatenate([w1_0, w1_1, w1_2], axis=1).astype(BF16)
    b1m = np.stack([b1_0, b1_1, b1_2], axis=1).astype(np.float32)
    ident = np.eye(P, dtype=BF16)

    in_maps = []
    for c in range(NCORES):
        rows = slice(c * NSH, (c + 1) * NSH)
        dm = dinv_slot[rows].reshape(BPC, P).T.copy()
        in_maps.append(dict(
            u0=u0,
            u0own=u0[rows],
            xT=np.ascontiguousarray(x_slot[rows].T).astype(BF16),
            idx=_wrap_idx(sidx_pad[c]),
            ldst=np.ascontiguousarray(ldst_all[c].reshape(TOTCH, P).T),
            iota=np.tile(np.arange(P, dtype=np.float32), (P, 1)).astype(BF16),
            dinvc=dm,
            dinv2c=(dm * dm),
            ident=ident,
            w1s=w1s,
            b1m=b1m,
            w2a=np.asarray(w2_0, np.float32).astype(BF16),
            w2b=np.asarray(w2_1, np.float32).astype(BF16),
            brow=brow,
        ))
    return in_maps, node2slot, tuple(int(v) for v in K_lo), \
        tuple(int(v) for v in K_hi)


# --------------------------------------------------------------------------
# device program
# --------------------------------------------------------------------------
def _build(nc, K_lo, K_hi, stage="full"):
    dt = mybir.dt
    f32 = dt.float32
    bf16 = dt.bfloat16
    Ktot = [a + b for a, b in zip(K_lo, K_hi)]
    TOTCH = sum(Ktot)
    KLO_M, KHI_M, KT_M = max(K_lo), max(K_hi), max(Ktot)
    cbase = [0]
    for k in Ktot:
        cbase.append(cbase[-1] + k)

    u0 = nc.dram_tensor("u0", [S, FIN], bf16, kind="ExternalInput").ap()
    u0own = nc.dram_tensor("u0own", [NSH, FIN], bf16, kind="ExternalInput").ap()
    xT = nc.dram_tensor("xT", [P, NSH], bf16, kind="ExternalInput").ap()
    idx = nc.dram_tensor("idx", [P, TOTCH * 8], dt.int16, kind="ExternalInput").ap()
    ldst = nc.dram_tensor("ldst", [P, TOTCH], f32, kind="ExternalInput").ap()
    iota = nc.dram_tensor("iota", [P, P], bf16, kind="ExternalInput").ap()
    dinvc = nc.dram_tensor("dinvc", [P, BPC], f32, kind="ExternalInput").ap()
    dinv2c = nc.dram_tensor("dinv2c", [P, BPC], f32, kind="ExternalInput").ap()
    identd = nc.dram_tensor("ident", [P, P], bf16, kind="ExternalInput").ap()
    w1s = nc.dram_tensor("w1s", [P, 3 * H], bf16, kind="ExternalInput").ap()
    b1m = nc.dram_tensor("b1m", [P, 3], f32, kind="ExternalInput").ap()
    w2a = nc.dram_tensor("w2a", [3 * H, CO], bf16, kind="ExternalInput").ap()
    w2b = nc.dram_tensor("w2b", [3 * H, CO], bf16, kind="ExternalInput").ap()
    brow = nc.dram_tensor("brow", [P, 2 * CO], f32, kind="ExternalInput").ap()
    out = nc.dram_tensor("out", [NSH, 2 * CO], f32, kind="ExternalOutput").ap()

    rg = [list(range(NCORES))]

    with tile.TileContext(nc) as tc:
        with (
            tc.tile_pool(name="res", bufs=1) as res,
            tc.tile_pool(name="dram", bufs=1, space="DRAM") as dram,
        ):
            def load(name, src_ap, shape, dtype=f32):
                t = res.tile(shape, dtype, tag=name, name=name)
                nc.sync.dma_start(out=t[:], in_=src_ap)
                return t

            idx_t = load("idx", idx[:], [P, TOTCH * 8], dt.int16)
            ldst_t = load("ldst", ldst[:], [P, TOTCH])
            iota_t = load("iota", iota[:], [P, P], bf16)
            dinvc_t = load("dinvc", dinvc[:], [P, BPC])
            dinv2c_t = load("dinv2c", dinv2c[:], [P, BPC])
            ident_t = load("ident", identd[:], [P, P], bf16)
            w1_t = load("w1s", w1s[:], [P, 3 * H], bf16)
            b1_t = load("b1m", b1m[:], [P, 3])
            brow_t = load("brow", brow[:], [P, 2 * CO])
            w2a_t = [load(f"w2a{i}", w2a[i * H:(i + 1) * H, :], [P, CO], bf16)
                     for i in range(3)]
            w2b_t = [load(f"w2b{i}", w2b[i * H:(i + 1) * H, :], [P, CO], bf16)
                     for i in range(3)]

            x1T = res.tile([P, NSH], bf16, tag="x1T")
            x2T = res.tile([P, NSH], bf16, tag="x2T")
            hT = [res.tile([P, NSH], bf16, tag=f"hT{i}", name=f"hT{i}")
                  for i in range(3)]
            out80 = res.tile([P, BPC * 2 * CO], f32, tag="out80")

            u1b = dram.tile([NSH, FIN], bf16, tag="u1b")
            u1f = dram.tile([S, FIN], bf16, tag="u1f", addr_space="Shared")
            uzb = dram.tile([NSH, P], bf16, tag="uzb")
            uzf = dram.tile([S, P], bf16, tag="uzf", addr_space="Shared")

            qn = [0]  # SWDGE queue round-robin

            def gather_half(pw, tag, tbl, b, Ks, base_off, KM):
                Kp = Ks[b]
                v = pw.tile([P, KM, FIN], bf16, tag=tag, name=tag)
                o = base_off[b]
                nc.gpsimd.dma_gather(
                    v[:, 0:Kp, :], tbl, idx_t[:, o * 8:(o + Kp) * 8],
                    num_idxs=Kp * P, num_idxs_reg=Kp * P, elem_size=FIN,
                    queue_num=qn[0])
                qn[0] = (qn[0] + 1) % NQ
                return v

            base_lo = [cbase[b] for b in range(BPC)]
            base_hi = [cbase[b] + K_lo[b] for b in range(BPC)]

            def prop(tbl, own, width, evict, pools):
                pw, pp, ohp, sp = pools
                for b in range(BPC):
                    vlo = gather_half(pw, "vlo", tbl[0:HALF, :], b, K_lo,
                                      base_lo, KLO_M)
                    vhi = gather_half(pw, "vhi", tbl[HALF:S, :], b, K_hi,
                                      base_hi, KHI_M)
                    ohts = []
                    for j in range(Ktot[b]):
                        ot = ohp.tile([P, P], bf16, tag="oh1")
                        nc.vector.tensor_scalar(
                            out=ot[:], in0=iota_t[:],
                            scalar1=ldst_t[:, cbase[b] + j:cbase[b] + j + 1],
                            scalar2=None, op0=mybir.AluOpType.is_equal)
                        ohts.append(ot)
                    sblk = sp.tile([P, FIN], bf16, tag="sblk")
                    nc.sync.dma_start(out=sblk[:],
                                      in_=own[b * P:(b + 1) * P, :])
                    ps = pp.tile([P, width], f32, tag="agg")
                    nc.tensor.matmul(out=ps[:], lhsT=ident_t[:],
                                     rhs=sblk[:, 0:width],
                                     start=True, stop=False)
                    for j in range(Ktot[b]):
                        srcv = (vlo[:, j, 0:width] if j < K_lo[b]
                                else vhi[:, j - K_lo[b], 0:width])
                        nc.tensor.matmul(
                            out=ps[:],
                            lhsT=ohts[j][:],
                            rhs=srcv,
                            start=False, stop=(j == Ktot[b] - 1))
                    evict(b, ps)

            # ================= P1: x1 = A x =================
            with (
                tc.tile_pool(name="p1w", bufs=3) as pw,
                tc.tile_pool(name="p1p", bufs=4, space="PSUM") as pp,
                tc.tile_pool(name="p1o", bufs=32) as ohp,
                tc.tile_pool(name="p1s", bufs=2) as sp,
                tc.tile_pool(name="p1e", bufs=3) as evp,
                tc.tile_pool(name="p1t", bufs=2, space="PSUM") as tpp,
            ):
                def evict1(b, ps):
                    x1t = evp.tile([P, P], bf16, tag="x1t")
                    nc.scalar.mul(x1t[:], ps[:], dinvc_t[:, b:b + 1])
                    u1t = evp.tile([P, P], bf16, tag="u1t")
                    nc.scalar.mul(u1t[:], ps[:], dinv2c_t[:, b:b + 1])
                    nc.sync.dma_start(out=u1b[b * P:(b + 1) * P, :], in_=u1t[:])
                    trp = tpp.tile([P, P], bf16, tag="trp")
                    nc.tensor.transpose(out=trp[:], in_=x1t[:],
                                        identity=ident_t[:])
                    nc.vector.tensor_copy(out=x1T[:, b * P:(b + 1) * P],
                                          in_=trp[:])

                prop(u0, u0own, FIN, evict1, (pw, pp, ohp, sp))

            nc.gpsimd.collective_compute(
                "AllGather", mybir.AluOpType.bypass, replica_groups=rg,
                ins=[u1b.opt()], outs=[u1f.opt()])

            if stage == "p1":
                dbg = nc.dram_tensor("dbg", [S, FIN], f32,
                                     kind="ExternalOutput").ap()
                with tc.tile_pool(name="dbgp", bufs=2) as dp:
                    for r0 in range(0, S, P):
                        t = dp.tile([P, FIN], f32, tag="dbgt")
                        nc.vector.tensor_copy(out=t[:], in_=u1f[r0:r0 + P, :])
                        nc.sync.dma_start(out=dbg[r0:r0 + P, :], in_=t[:])
                return

            # ================= P2: x2 = A x1 =================
            with (
                tc.tile_pool(name="p2w", bufs=3) as pw,
                tc.tile_pool(name="p2p", bufs=4, space="PSUM") as pp,
                tc.tile_pool(name="p2o", bufs=32) as ohp,
                tc.tile_pool(name="p2s", bufs=2) as sp,
                tc.tile_pool(name="p2e", bufs=3) as evp,
                tc.tile_pool(name="p2t", bufs=2, space="PSUM") as tpp,
            ):
                def evict2(b, ps):
                    x2t = evp.tile([P, P], bf16, tag="x2t")
                    nc.scalar.mul(x2t[:], ps[:], dinvc_t[:, b:b + 1])
                    trp = tpp.tile([P, P], bf16, tag="trp2")
                    nc.tensor.transpose(out=trp[:], in_=x2t[:],
                                        identity=ident_t[:])
                    nc.vector.tensor_copy(out=x2T[:, b * P:(b + 1) * P],
                                          in_=trp[:])

                prop(u1f, u1b, FIN, evict2, (pw, pp, ohp, sp))

            if stage == "p2":
                dbg = nc.dram_tensor("dbg", [2 * P, NSH], f32,
                                     kind="ExternalOutput").ap()
                t = res.tile([P, NSH], f32, tag="dbgt")
                nc.vector.tensor_copy(out=t[:], in_=x1T[:])
                nc.sync.dma_start(out=dbg[0:P, :], in_=t[:])
                nc.vector.tensor_copy(out=t[:], in_=x2T[:])
                nc.sync.dma_start(out=dbg[P:2 * P, :], in_=t[:])
                return

            # ================= dense: hT = relu(w1^T [x|x1|x2]^T + b1) ======
            with (
                tc.tile_pool(name="dxs", bufs=3) as xsp,
                tc.tile_pool(name="dps", bufs=3, space="PSUM") as hpp,
            ):
                for f0 in range(0, NSH, 512):
                    w = min(512, NSH - f0)
                    xt = xsp.tile([P, 512], bf16, tag="xs")
                    nc.sync.dma_start(out=xt[:, 0:w], in_=xT[:, f0:f0 + w])
                    srcs = (xt[:, 0:w], x1T[:, f0:f0 + w], x2T[:, f0:f0 + w])
                    for i in range(3):
                        ph = hpp.tile([P, 512], f32, tag="hps")
                        nc.tensor.matmul(out=ph[:, 0:w],
                                         lhsT=w1_t[:, i * H:(i + 1) * H],
                                         rhs=srcs[i], start=True, stop=True)
                        nc.scalar.activation(
                            out=hT[i][:, f0:f0 + w], in_=ph[:, 0:w],
                            func=mybir.ActivationFunctionType.Relu,
                            bias=b1_t[:, i:i + 1], scale=1.0)

            if stage == "dense":
                dbg = nc.dram_tensor("dbg", [3 * P, NSH], f32,
                                     kind="ExternalOutput").ap()
                t = res.tile([P, NSH], f32, tag="dbgt")
                for i in range(3):
                    nc.vector.tensor_copy(out=t[:], in_=hT[i][:])
                    nc.sync.dma_start(out=dbg[i * P:(i + 1) * P, :], in_=t[:])
                return

            # ================= z1 = dinv*(h w2_1) (-> uz), out1 = h w2_0 ====
            with (
                tc.tile_pool(name="eps", bufs=4, space="PSUM") as zpp,
                tc.tile_pool(name="eev", bufs=3) as evp,
            ):
                for b in range(BPC):
                    pz = zpp.tile([P, CO], f32, tag="pz")
                    for i in range(3):
                        nc.tensor.matmul(out=pz[:],
                                         lhsT=hT[i][:, b * P:(b + 1) * P],
                                         rhs=w2b_t[i][:], start=(i == 0),
                                         stop=(i == 2))
                    uzt = evp.tile([P, P], bf16, tag="uzt")
                    nc.vector.memset(uzt[:, CO:P], 0)
                    nc.scalar.mul(uzt[:, 0:CO], pz[:], dinvc_t[:, b:b + 1])
                    nc.sync.dma_start(out=uzb[b * P:(b + 1) * P, :], in_=uzt[:])
                    po = zpp.tile([P, CO], f32, tag="po")
                    for i in range(3):
                        nc.tensor.matmul(out=po[:],
                                         lhsT=hT[i][:, b * P:(b + 1) * P],
                                         rhs=w2a_t[i][:], start=(i == 0),
                                         stop=(i == 2))
                    nc.vector.tensor_copy(
                        out=out80[:, b * 2 * CO:b * 2 * CO + CO], in_=po[:])

            nc.gpsimd.collective_compute(
                "AllGather", mybir.AluOpType.bypass, replica_groups=rg,
                ins=[uzb.opt()], outs=[uzf.opt()])

            # ========== P3: out2 = dinv * A' z1, fused bias+softmax-head ====
            # Per-block softmax prologue (bias, max, shift, exp+accum) rides
            # inside the gather-bound P3 loop; the Ln over all 57 row-sums is
            # ONE activation at the end (the ACT engine reloads its function
            # table on Exp<->Ln switches, 1.3us each -- batching avoids 2*57
            # reloads).
            sh_all = res.tile([P, BPC * 2 * CO], f32, tag="sh_all")
            se_all = res.tile([P, BPC], f32, tag="se_all")
            LAG = 6  # softmax prologue trails the prop by LAG blocks so the
            #          exp on ACT never head-of-line-blocks the PSUM-releasing
            #          mul of the current block.
            with (
                tc.tile_pool(name="p3w", bufs=3) as pw,
                tc.tile_pool(name="p3p", bufs=4, space="PSUM") as pp,
                tc.tile_pool(name="p3o", bufs=32) as ohp,
                tc.tile_pool(name="p3s", bufs=2) as sp,
                tc.tile_pool(name="p3f", bufs=3) as fp,
            ):
                def smax_head(b):
                    t1 = fp.tile([P, 2 * CO], f32, tag="f1")
                    nc.vector.tensor_tensor(
                        out=t1[:], in0=out80[:, b * 2 * CO:(b + 1) * 2 * CO],
                        in1=brow_t[:], op=mybir.AluOpType.add)
                    mx = fp.tile([P, 1], f32, tag="mx")
                    nc.vector.reduce_max(out=mx[:], in_=t1[:],
                                         axis=mybir.AxisListType.X)
                    sh = sh_all[:, b * 2 * CO:(b + 1) * 2 * CO]
                    nc.vector.tensor_scalar(out=sh, in0=t1[:], scalar1=mx[:],
                                            scalar2=None,
                                            op0=mybir.AluOpType.subtract)
                    ex = fp.tile([P, 2 * CO], f32, tag="ex")
                    nc.scalar.activation(out=ex[:], in_=sh,
                                         func=mybir.ActivationFunctionType.Exp,
                                         accum_out=se_all[:, b:b + 1])

                def evict3(b, ps):
                    nc.scalar.mul(
                        out80[:, b * 2 * CO + CO:(b + 1) * 2 * CO], ps[:],
                        dinvc_t[:, b:b + 1])
                    if b >= LAG:
                        smax_head(b - LAG)

                prop(uzf, uzb, CO, evict3, (pw, pp, ohp, sp))
                for b in range(BPC - LAG, BPC):
                    smax_head(b)

            # ================= lse + final subtract + store ==================
            with tc.tile_pool(name="fin", bufs=3) as fp:
                lse = res.tile([P, BPC], f32, tag="lse")
                nc.scalar.activation(out=lse[:], in_=se_all[:],
                                     func=mybir.ActivationFunctionType.Ln)
                for b in range(BPC):
                    r = fp.tile([P, 2 * CO], f32, tag="r")
                    nc.vector.tensor_scalar(
                        out=r[:], in0=sh_all[:, b * 2 * CO:(b + 1) * 2 * CO],
                        scalar1=lse[:, b:b + 1], scalar2=None,
                        op0=mybir.AluOpType.subtract)
                    nc.sync.dma_start(out=out[b * P:(b + 1) * P, :], in_=r[:])


def _get_compiled(K_lo, K_hi, stage="full"):
    key = (K_lo, K_hi, stage)
    if key not in _COMPILED:
        nc = bacc.Bacc("TRN2", target_bir_lowering=False, debug=False,
                       num_devices=NCORES, num_swdge_queues=NQ)
        _build(nc, K_lo, K_hi, stage=stage)
        nc.compile()
        _COMPILED[key] = nc
    return _COMPILED[key]


def kernel(**inputs):
    global LAST_RESULT
    args = {k: np.asarray(v) for k, v in inputs.items()}
    in_maps, node2slot, K_lo, K_hi = _preprocess(
        args["x"].astype(np.float32), args["edge_index"],
        args["w1_0"].astype(np.float32), args["b1_0"].astype(np.float32),
        args["w1_1"].astype(np.float32), args["b1_1"].astype(np.float32),
        args["w1_2"].astype(np.float32), args["b1_2"].astype(np.float32),
        args["w2_0"].astype(np.float32), args["b2_0"].astype(np.float32),
        args["w2_1"].astype(np.float32), args["b2_1"].astype(np.float32),
    )
    nc = _get_compiled(K_lo, K_hi)
    res = run_bass_kernel_spmd(nc, in_maps, list(range(NCORES)))
    LAST_RESULT = res
    out_slot = np.concatenate([res.results[c]["out"] for c in range(NCORES)],
                              axis=0)
    return out_slot[node2slot].astype(np.float32)


# revision 11
# speedup vs baseline: 1.3717x; 1.3583x over previous
"""MixHopNet GNN kernel for 8 Trainium2 NeuronCores (Bass/Tile SPMD).

Math (reference): GCN-normalized adjacency A = D^-1/2 (Adj + I) D^-1/2 over
N=50000 nodes / 800k random edges (+self loops), then
  x1 = A x ; x2 = A x1
  h  = relu([x w1_0 + b1_0, x1 w1_1 + b1_1, x2 w1_2 + b1_2])
  out = log_softmax([h w2_0 + b2_0, (A h) w2_1 + b2_1])

Distribution (graph/data parallel): nodes are packed into 456 blocks of 128
slots (degree-balanced bin packing), 57 blocks per core.  Propagation for a
dst block accumulates S_j^T @ V_j over edge chunks j of 128 edges in PSUM,
where V_j is a dma_gather of the bf16 source-row table and S_j is a PURE 0/1
one-hot (edge -> dst slot) PRECOMPUTED ON HOST and streamed from HBM as bf16
(on-HW DVE tensor_scalar one-hot builds cost ~1.7us each; streaming is ~free).
The two D^-1/2 factors: gather tables are pre-scaled by dinv[src], and
dinv[dst] is applied once per block at PSUM eviction (it factors out of the
edge sum).  Reference self-loops are NOT in the edge list; they are the
identity matmul of the block's own (locally available) table rows.

dma_gather descriptor generation is the critical resource (~7ns/descriptor,
measured); calls are spread round-robin over all 4 SWDGE queues and use
exact per-(block,half) chunk counts padded to the max over cores at the same
sorted block position (so the SPMD program is identical on every core).

Cross-core: tables are replicated; x1 and z1 = dinv*(h @ w2_1) shards are
AllGather'd between propagations ((A h) w2_1 == A (h w2_1), so only 40 cols
propagate in conv2).  dma_gather needs int16 indices, so tables are split in
two 29184-row halves aligned with the node-id split.
"""
import sys

sys.path.insert(0, "/opt/trn_rl_repo")

import numpy as np

import concourse.bass as bass  # noqa: F401
import concourse.bacc as bacc
import concourse.tile as tile
from concourse import mybir
from concourse.bass_utils import run_bass_kernel_spmd

import ml_dtypes

BF16 = ml_dtypes.bfloat16

# ---- problem constants (hardcoded; kernel.py must be self-contained) ----
N = 50000
FIN = 128
H = 128
CO = 40
NCORES = 8
P = 128
NB = 456               # node blocks total
BPC = NB // NCORES     # 57 blocks per core
S = NB * P             # 58368 slots
NSH = BPC * P          # 7296 slots per core
HALF = S // 2          # 29184  (int16-addressable table half)
NHALF = N // 2         # node-id split matching the slot-half split
NQ = 4                 # SWDGE queues

LAST_RESULT = None     # BassKernelResults of the most recent run (for test.py)

_COMPILED = {}


# --------------------------------------------------------------------------
# host-side preprocessing
# --------------------------------------------------------------------------
def _pack_nodes(a, b):
    """Assign each node a slot: nodes [0,NHALF) -> blocks [0,NB/2), rest ->
    blocks [NB/2,NB).  Greedy 2D bin packing (descending total degree,
    minimize max(lo_load, hi_load)) balances both src-half in-edge counts."""
    node2slot = np.empty(N, np.int64)
    for hstart, hend, b0 in ((0, NHALF, 0), (NHALF, N, NB // 2)):
        nbins = NB // 2
        nodes = np.arange(hstart, hend)
        nodes = nodes[np.argsort(-(a[nodes] + b[nodes]), kind="stable")]
        lo_load = np.zeros(nbins, np.int64)
        hi_load = np.zeros(nbins, np.int64)
        cnt = np.zeros(nbins, np.int64)
        av = a[nodes]
        bv = b[nodes]
        for i in range(nodes.shape[0]):
            score = np.maximum(lo_load + av[i], hi_load + bv[i])
            score[cnt >= P] = 1 << 60
            blk = int(np.argmin(score))
            node2slot[nodes[i]] = (b0 + blk) * P + cnt[blk]
            cnt[blk] += 1
            lo_load[blk] += av[i]
            hi_load[blk] += bv[i]
    return node2slot


def _wrap_idx(flat):
    """[n*128] int16 -> [128, n*8] (16-partition wrap, replicated 8x)."""
    n = flat.shape[0] // 128
    arr = flat.reshape(n * 8, 16).T.copy()
    return np.tile(arr, (8, 1))


def _preprocess(x, edge_index, w1_0, b1_0, w1_1, b1_1, w1_2, b1_2,
                w2_0, b2_0, w2_1, b2_1):
    src = edge_index[0].astype(np.int64)
    dst = edge_index[1].astype(np.int64)
    E = src.shape[0]

    deg = (np.bincount(dst, minlength=N) + 1).astype(np.float32)  # +self loop
    dinv = (1.0 / np.sqrt(deg)).astype(np.float32)

    islo_n = src < NHALF
    a = np.bincount(dst[islo_n], minlength=N)
    b = np.bincount(dst[~islo_n], minlength=N)
    node2slot = _pack_nodes(a, b)

    # per-(old block, src half) edge counts, to sort/deal blocks to cores
    blk_old = node2slot[dst] >> 7
    grp_old = blk_old * 2 + (~islo_n).astype(np.int64)
    cnts = np.bincount(grp_old, minlength=2 * NB).reshape(NB, 2)
    tot = cnts.sum(axis=1)
    # block -> position: snake-deal blocks (sorted by total edges desc)
    # within each half so every core gets a similar block-size profile.
    order_bs = np.empty(NB, np.int64)  # new position -> old block id
    blk_perm = np.empty(NB, np.int64)  # old block id -> new position
    for half, coff in ((0, 0), (1, 4)):
        ids = np.arange(half * (NB // 2), (half + 1) * (NB // 2))
        ids = ids[np.argsort(-tot[ids], kind="stable")]
        percore = [[] for _ in range(4)]
        for i, bid in enumerate(ids):
            c = i % 4 if (i // 4) % 2 == 0 else 3 - i % 4
            percore[c].append(bid)
        for c in range(4):
            for j, bid in enumerate(percore[c]):
                pos = (coff + c) * BPC + j
                order_bs[pos] = bid
                blk_perm[bid] = pos
    slot_perm = (blk_perm[:, None] * P + np.arange(P)[None, :]).reshape(-1)
    node2slot = slot_perm[node2slot]
    cnts = cnts[order_bs]              # [position, half] edge counts

    dslot = node2slot[dst]
    nblk = dslot >> 7                  # block position (0..NB-1)
    d_in_blk = dslot & 127
    sslot = node2slot[src]
    islo = sslot < HALF                # == islo_n (halves preserved)
    sidx = np.where(islo, sslot, sslot - HALF).astype(np.int16)

    # chunk counts per (within-core position, half): max across cores
    ch = np.ceil(cnts / P).astype(np.int64).reshape(NCORES, BPC, 2)
    K_lo = np.maximum(ch[:, :, 0].max(axis=0), 1)  # [BPC]
    K_hi = np.maximum(ch[:, :, 1].max(axis=0), 1)
    Ktot = K_lo + K_hi
    TOTCH = int(Ktot.sum())            # chunks per core

    # flatten edges into the padded chunk layout
    grp = nblk * 2 + (~islo).astype(np.int64)
    order = np.argsort(grp, kind="stable")
    gs = np.bincount(grp, minlength=2 * NB)
    starts = np.concatenate([[0], np.cumsum(gs)[:-1]])
    epos = np.arange(E) - starts[grp[order]]    # position within group

    base_lo = np.concatenate([[0], np.cumsum(Ktot)[:-1]])  # [BPC] chunk base
    base_hi = base_lo + K_lo
    posc = np.arange(NB) % BPC
    gbase = np.stack([base_lo[posc], base_hi[posc]], axis=1)  # [NB, 2]
    eslot = gbase[nblk[order], (~islo[order]).astype(np.int64)] * P + epos
    core_e = nblk[order] // BPC

    sidx_pad = np.zeros((NCORES, TOTCH * P), np.int16)
    sidx_pad[core_e, eslot] = sidx[order]
    oh = np.zeros((NCORES, TOTCH * P, P), np.float32)
    oh[core_e, eslot, d_in_blk[order]] = 1.0

    dinv_slot = np.zeros(S, np.float32)
    dinv_slot[node2slot] = dinv
    x_slot = np.zeros((S, FIN), np.float32)
    x_slot[node2slot] = x
    u0 = (x_slot * dinv_slot[:, None]).astype(BF16)

    brow = np.tile(np.concatenate([b2_0, b2_1])[None, :], (P, 1)).astype(np.float32)
    w1s = np.concatenate([w1_0, w1_1, w1_2], axis=1).astype(BF16)
    b1m = np.stack([b1_0, b1_1, b1_2], axis=1).astype(np.float32)
    ident = np.eye(P, dtype=BF16)

    in_maps = []
    for c in range(NCORES):
        rows = slice(c * NSH, (c + 1) * NSH)
        dm = dinv_slot[rows].reshape(BPC, P).T.copy()
        # device one-hot layout: [128 edge-row, TOTCH*128 (chunk, dst)]
        ohc = np.ascontiguousarray(
            oh[c].reshape(TOTCH, P, P).transpose(1, 0, 2)
            .reshape(P, TOTCH * P)).astype(BF16)
        in_maps.append(dict(
            u0=u0,
            u0own=u0[rows],
            xT=np.ascontiguousarray(x_slot[rows].T).astype(BF16),
            idx=_wrap_idx(sidx_pad[c]),
            oh=ohc,
            dinvc=dm,
            dinv2c=(dm * dm),
            ident=ident,
            w1s=w1s,
            b1m=b1m,
            w2a=np.asarray(w2_0, np.float32).astype(BF16),
            w2b=np.asarray(w2_1, np.float32).astype(BF16),
            brow=brow,
        ))
    return in_maps, node2slot, tuple(int(v) for v in K_lo), \
        tuple(int(v) for v in K_hi)


# --------------------------------------------------------------------------
# device program
# --------------------------------------------------------------------------
def _build(nc, K_lo, K_hi, stage="full"):
    dt = mybir.dt
    f32 = dt.float32
    bf16 = dt.bfloat16
    Ktot = [a + b for a, b in zip(K_lo, K_hi)]
    TOTCH = sum(Ktot)
    KLO_M, KHI_M, KT_M = max(K_lo), max(K_hi), max(Ktot)
    cbase = [0]
    for k in Ktot:
        cbase.append(cbase[-1] + k)

    u0 = nc.dram_tensor("u0", [S, FIN], bf16, kind="ExternalInput").ap()
    u0own = nc.dram_tensor("u0own", [NSH, FIN], bf16, kind="ExternalInput").ap()
    xT = nc.dram_tensor("xT", [P, NSH], bf16, kind="ExternalInput").ap()
    idx = nc.dram_tensor("idx", [P, TOTCH * 8], dt.int16, kind="ExternalInput").ap()
    ohd = nc.dram_tensor("oh", [P, TOTCH * P], bf16, kind="ExternalInput").ap()
    dinvc = nc.dram_tensor("dinvc", [P, BPC], f32, kind="ExternalInput").ap()
    dinv2c = nc.dram_tensor("dinv2c", [P, BPC], f32, kind="ExternalInput").ap()
    identd = nc.dram_tensor("ident", [P, P], bf16, kind="ExternalInput").ap()
    w1s = nc.dram_tensor("w1s", [P, 3 * H], bf16, kind="ExternalInput").ap()
    b1m = nc.dram_tensor("b1m", [P, 3], f32, kind="ExternalInput").ap()
    w2a = nc.dram_tensor("w2a", [3 * H, CO], bf16, kind="ExternalInput").ap()
    w2b = nc.dram_tensor("w2b", [3 * H, CO], bf16, kind="ExternalInput").ap()
    brow = nc.dram_tensor("brow", [P, 2 * CO], f32, kind="ExternalInput").ap()
    out = nc.dram_tensor("out", [NSH, 2 * CO], f32, kind="ExternalOutput").ap()

    rg = [list(range(NCORES))]

    with tile.TileContext(nc) as tc:
        with (
            tc.tile_pool(name="res", bufs=1) as res,
            tc.tile_pool(name="dram", bufs=1, space="DRAM") as dram,
        ):
            def load(name, src_ap, shape, dtype=f32):
                t = res.tile(shape, dtype, tag=name, name=name)
                nc.sync.dma_start(out=t[:], in_=src_ap)
                return t

            idx_t = load("idx", idx[:], [P, TOTCH * 8], dt.int16)
            dinvc_t = load("dinvc", dinvc[:], [P, BPC])
            dinv2c_t = load("dinv2c", dinv2c[:], [P, BPC])
            ident_t = load("ident", identd[:], [P, P], bf16)
            w1_t = load("w1s", w1s[:], [P, 3 * H], bf16)
            b1_t = load("b1m", b1m[:], [P, 3])
            brow_t = load("brow", brow[:], [P, 2 * CO])
            w2a_t = [load(f"w2a{i}", w2a[i * H:(i + 1) * H, :], [P, CO], bf16)
                     for i in range(3)]
            w2b_t = [load(f"w2b{i}", w2b[i * H:(i + 1) * H, :], [P, CO], bf16)
                     for i in range(3)]

            x1T = res.tile([P, NSH], bf16, tag="x1T")
            x2T = res.tile([P, NSH], bf16, tag="x2T")
            hT = [res.tile([P, NSH], bf16, tag=f"hT{i}", name=f"hT{i}")
                  for i in range(3)]
            out80 = res.tile([P, BPC * 2 * CO], f32, tag="out80")

            u1b = dram.tile([NSH, FIN], bf16, tag="u1b")
            u1f = dram.tile([S, FIN], bf16, tag="u1f", addr_space="Shared")
            uzb = dram.tile([NSH, P], bf16, tag="uzb")
            uzf = dram.tile([S, P], bf16, tag="uzf", addr_space="Shared")

            qn = [0]  # SWDGE queue round-robin

            def gather_half(pw, tag, tbl, b, Ks, base_off, KM):
                Kp = Ks[b]
                v = pw.tile([P, KM, FIN], bf16, tag=tag, name=tag)
                o = base_off[b]
                nc.gpsimd.dma_gather(
                    v[:, 0:Kp, :], tbl, idx_t[:, o * 8:(o + Kp) * 8],
                    num_idxs=Kp * P, num_idxs_reg=Kp * P, elem_size=FIN,
                    queue_num=qn[0])
                qn[0] = (qn[0] + 1) % NQ
                return v

            base_lo = [cbase[b] for b in range(BPC)]
            base_hi = [cbase[b] + K_lo[b] for b in range(BPC)]

            def prop(tbl, own, width, evict, pools):
                pw, pp, ohp, sp = pools
                for b in range(BPC):
                    vlo = gather_half(pw, "vlo", tbl[0:HALF, :], b, K_lo,
                                      base_lo, KLO_M)
                    vhi = gather_half(pw, "vhi", tbl[HALF:S, :], b, K_hi,
                                      base_hi, KHI_M)
                    oht = ohp.tile([P, KT_M * P], bf16, tag="oht")
                    nc.sync.dma_start(
                        out=oht[:, 0:Ktot[b] * P],
                        in_=ohd[:, cbase[b] * P:cbase[b + 1] * P])
                    sblk = sp.tile([P, FIN], bf16, tag="sblk")
                    nc.sync.dma_start(out=sblk[:],
                                      in_=own[b * P:(b + 1) * P, :])
                    ps = pp.tile([P, width], f32, tag="agg")
                    nc.tensor.matmul(out=ps[:], lhsT=ident_t[:],
                                     rhs=sblk[:, 0:width],
                                     start=True, stop=False)
                    for j in range(Ktot[b]):
                        srcv = (vlo[:, j, 0:width] if j < K_lo[b]
                                else vhi[:, j - K_lo[b], 0:width])
                        nc.tensor.matmul(
                            out=ps[:],
                            lhsT=oht[:, j * P:(j + 1) * P],
                            rhs=srcv,
                            start=False, stop=(j == Ktot[b] - 1))
                    evict(b, ps)

            # ================= P1: x1 = A x =================
            with (
                tc.tile_pool(name="p1w", bufs=3) as pw,
                tc.tile_pool(name="p1p", bufs=4, space="PSUM") as pp,
                tc.tile_pool(name="p1o", bufs=3) as ohp,
                tc.tile_pool(name="p1s", bufs=2) as sp,
                tc.tile_pool(name="p1e", bufs=3) as evp,
                tc.tile_pool(name="p1t", bufs=2, space="PSUM") as tpp,
            ):
                def evict1(b, ps):
                    x1t = evp.tile([P, P], bf16, tag="x1t")
                    nc.scalar.mul(x1t[:], ps[:], dinvc_t[:, b:b + 1])
                    u1t = evp.tile([P, P], bf16, tag="u1t")
                    nc.scalar.mul(u1t[:], ps[:], dinv2c_t[:, b:b + 1])
                    nc.sync.dma_start(out=u1b[b * P:(b + 1) * P, :], in_=u1t[:])
                    trp = tpp.tile([P, P], bf16, tag="trp")
                    nc.tensor.transpose(out=trp[:], in_=x1t[:],
                                        identity=ident_t[:])
                    nc.vector.tensor_copy(out=x1T[:, b * P:(b + 1) * P],
                                          in_=trp[:])

                prop(u0, u0own, FIN, evict1, (pw, pp, ohp, sp))

            nc.gpsimd.collective_compute(
                "AllGather", mybir.AluOpType.bypass, replica_groups=rg,
                ins=[u1b.opt()], outs=[u1f.opt()])

            if stage == "p1":
                dbg = nc.dram_tensor("dbg", [S, FIN], f32,
                                     kind="ExternalOutput").ap()
                with tc.tile_pool(name="dbgp", bufs=2) as dp:
                    for r0 in range(0, S, P):
                        t = dp.tile([P, FIN], f32, tag="dbgt")
                        nc.vector.tensor_copy(out=t[:], in_=u1f[r0:r0 + P, :])
                        nc.sync.dma_start(out=dbg[r0:r0 + P, :], in_=t[:])
                return

            # ================= P2: x2 = A x1 =================
            with (
                tc.tile_pool(name="p2w", bufs=3) as pw,
                tc.tile_pool(name="p2p", bufs=4, space="PSUM") as pp,
                tc.tile_pool(name="p2o", bufs=3) as ohp,
                tc.tile_pool(name="p2s", bufs=2) as sp,
                tc.tile_pool(name="p2e", bufs=3) as evp,
                tc.tile_pool(name="p2t", bufs=2, space="PSUM") as tpp,
            ):
                def evict2(b, ps):
                    x2t = evp.tile([P, P], bf16, tag="x2t")
                    nc.scalar.mul(x2t[:], ps[:], dinvc_t[:, b:b + 1])
                    trp = tpp.tile([P, P], bf16, tag="trp2")
                    nc.tensor.transpose(out=trp[:], in_=x2t[:],
                                        identity=ident_t[:])
                    nc.vector.tensor_copy(out=x2T[:, b * P:(b + 1) * P],
                                          in_=trp[:])

                prop(u1f, u1b, FIN, evict2, (pw, pp, ohp, sp))

            if stage == "p2":
                dbg = nc.dram_tensor("dbg", [2 * P, NSH], f32,
                                     kind="ExternalOutput").ap()
                t = res.tile([P, NSH], f32, tag="dbgt")
                nc.vector.tensor_copy(out=t[:], in_=x1T[:])
                nc.sync.dma_start(out=dbg[0:P, :], in_=t[:])
                nc.vector.tensor_copy(out=t[:], in_=x2T[:])
                nc.sync.dma_start(out=dbg[P:2 * P, :], in_=t[:])
                return

            # ================= dense: hT = relu(w1^T [x|x1|x2]^T + b1) ======
            with (
                tc.tile_pool(name="dxs", bufs=3) as xsp,
                tc.tile_pool(name="dps", bufs=3, space="PSUM") as hpp,
            ):
                for f0 in range(0, NSH, 512):
                    w = min(512, NSH - f0)
                    xt = xsp.tile([P, 512], bf16, tag="xs")
                    nc.sync.dma_start(out=xt[:, 0:w], in_=xT[:, f0:f0 + w])
                    srcs = (xt[:, 0:w], x1T[:, f0:f0 + w], x2T[:, f0:f0 + w])
                    for i in range(3):
                        ph = hpp.tile([P, 512], f32, tag="hps")
                        nc.tensor.matmul(out=ph[:, 0:w],
                                         lhsT=w1_t[:, i * H:(i + 1) * H],
                                         rhs=srcs[i], start=True, stop=True)
                        nc.scalar.activation(
                            out=hT[i][:, f0:f0 + w], in_=ph[:, 0:w],
                            func=mybir.ActivationFunctionType.Relu,
                            bias=b1_t[:, i:i + 1], scale=1.0)

            if stage == "dense":
                dbg = nc.dram_tensor("dbg", [3 * P, NSH], f32,
                                     kind="ExternalOutput").ap()
                t = res.tile([P, NSH], f32, tag="dbgt")
                for i in range(3):
                    nc.vector.tensor_copy(out=t[:], in_=hT[i][:])
                    nc.sync.dma_start(out=dbg[i * P:(i + 1) * P, :], in_=t[:])
                return

            # ================= z1 = dinv*(h w2_1) (-> uz), out1 = h w2_0 ====
            with (
                tc.tile_pool(name="eps", bufs=4, space="PSUM") as zpp,
                tc.tile_pool(name="eev", bufs=3) as evp,
            ):
                for b in range(BPC):
                    pz = zpp.tile([P, CO], f32, tag="pz")
                    for i in range(3):
                        nc.tensor.matmul(out=pz[:],
                                         lhsT=hT[i][:, b * P:(b + 1) * P],
                                         rhs=w2b_t[i][:], start=(i == 0),
                                         stop=(i == 2))
                    uzt = evp.tile([P, P], bf16, tag="uzt")
                    nc.vector.memset(uzt[:, CO:P], 0)
                    nc.scalar.mul(uzt[:, 0:CO], pz[:], dinvc_t[:, b:b + 1])
                    nc.sync.dma_start(out=uzb[b * P:(b + 1) * P, :], in_=uzt[:])
                    po = zpp.tile([P, CO], f32, tag="po")
                    for i in range(3):
                        nc.tensor.matmul(out=po[:],
                                         lhsT=hT[i][:, b * P:(b + 1) * P],
                                         rhs=w2a_t[i][:], start=(i == 0),
                                         stop=(i == 2))
                    nc.vector.tensor_copy(
                        out=out80[:, b * 2 * CO:b * 2 * CO + CO], in_=po[:])

            nc.gpsimd.collective_compute(
                "AllGather", mybir.AluOpType.bypass, replica_groups=rg,
                ins=[uzb.opt()], outs=[uzf.opt()])

            # ========== P3: out2 = dinv * A' z1, fused bias+softmax-head ====
            # Per-block softmax prologue (bias, max, shift, exp+accum) rides
            # inside the gather-bound P3 loop; the Ln over all 57 row-sums is
            # ONE activation at the end (the ACT engine reloads its function
            # table on Exp<->Ln switches, 1.3us each -- batching avoids 2*57
            # reloads).
            sh_all = res.tile([P, BPC * 2 * CO], f32, tag="sh_all")
            se_all = res.tile([P, BPC], f32, tag="se_all")
            LAG = 6  # softmax prologue trails the prop by LAG blocks so the
            #          exp on ACT never head-of-line-blocks the PSUM-releasing
            #          mul of the current block.
            with (
                tc.tile_pool(name="p3w", bufs=3) as pw,
                tc.tile_pool(name="p3p", bufs=4, space="PSUM") as pp,
                tc.tile_pool(name="p3o", bufs=3) as ohp,
                tc.tile_pool(name="p3s", bufs=2) as sp,
                tc.tile_pool(name="p3f", bufs=3) as fp,
            ):
                def smax_head(b):
                    t1 = fp.tile([P, 2 * CO], f32, tag="f1")
                    nc.vector.tensor_tensor(
                        out=t1[:], in0=out80[:, b * 2 * CO:(b + 1) * 2 * CO],
                        in1=brow_t[:], op=mybir.AluOpType.add)
                    mx = fp.tile([P, 1], f32, tag="mx")
                    nc.vector.reduce_max(out=mx[:], in_=t1[:],
                                         axis=mybir.AxisListType.X)
                    sh = sh_all[:, b * 2 * CO:(b + 1) * 2 * CO]
                    nc.vector.tensor_scalar(out=sh, in0=t1[:], scalar1=mx[:],
                                            scalar2=None,
                                            op0=mybir.AluOpType.subtract)
                    ex = fp.tile([P, 2 * CO], f32, tag="ex")
                    nc.scalar.activation(out=ex[:], in_=sh,
                                         func=mybir.ActivationFunctionType.Exp,
                                         accum_out=se_all[:, b:b + 1])

                def evict3(b, ps):
                    nc.scalar.mul(
                        out80[:, b * 2 * CO + CO:(b + 1) * 2 * CO], ps[:],
                        dinvc_t[:, b:b + 1])
                    if b >= LAG:
                        smax_head(b - LAG)

                prop(uzf, uzb, CO, evict3, (pw, pp, ohp, sp))
                for b in range(BPC - LAG, BPC):
                    smax_head(b)

            # ================= lse + final subtract + store ==================
            with tc.tile_pool(name="fin", bufs=3) as fp:
                lse = res.tile([P, BPC], f32, tag="lse")
                nc.scalar.activation(out=lse[:], in_=se_all[:],
                                     func=mybir.ActivationFunctionType.Ln)
                for b in range(BPC):
                    r = fp.tile([P, 2 * CO], f32, tag="r")
                    nc.vector.tensor_scalar(
                        out=r[:], in0=sh_all[:, b * 2 * CO:(b + 1) * 2 * CO],
                        scalar1=lse[:, b:b + 1], scalar2=None,
                        op0=mybir.AluOpType.subtract)
                    nc.sync.dma_start(out=out[b * P:(b + 1) * P, :], in_=r[:])


def _get_compiled(K_lo, K_hi, stage="full"):
    key = (K_lo, K_hi, stage)
    if key not in _COMPILED:
        nc = bacc.Bacc("TRN2", target_bir_lowering=False, debug=False,
                       num_devices=NCORES, num_swdge_queues=NQ)
        _build(nc, K_lo, K_hi, stage=stage)
        nc.compile()
        _COMPILED[key] = nc
    return _COMPILED[key]


def kernel(**inputs):
    global LAST_RESULT
    args = {k: np.asarray(v) for k, v in inputs.items()}
    in_maps, node2slot, K_lo, K_hi = _preprocess(
        args["x"].astype(np.float32), args["edge_index"],
        args["w1_0"].astype(np.float32), args["b1_0"].astype(np.float32),
        args["w1_1"].astype(np.float32), args["b1_1"].astype(np.float32),
        args["w1_2"].astype(np.float32), args["b1_2"].astype(np.float32),
        args["w2_0"].astype(np.float32), args["b2_0"].astype(np.float32),
        args["w2_1"].astype(np.float32), args["b2_1"].astype(np.float32),
    )
    nc = _get_compiled(K_lo, K_hi)
    res = run_bass_kernel_spmd(nc, in_maps, list(range(NCORES)))
    LAST_RESULT = res
    out_slot = np.concatenate([res.results[c]["out"] for c in range(NCORES)],
                              axis=0)
    return out_slot[node2slot].astype(np.float32)


# revision 12
# speedup vs baseline: 1.6346x; 1.1916x over previous
"""MixHopNet GNN kernel for 8 Trainium2 NeuronCores (Bass/Tile SPMD).

Math (reference): GCN-normalized adjacency A = D^-1/2 (Adj + I) D^-1/2 over
N=50000 nodes / 800k random edges (+self loops), then
  x1 = A x ; x2 = A x1
  h  = relu([x w1_0 + b1_0, x1 w1_1 + b1_1, x2 w1_2 + b1_2])
  out = log_softmax([h w2_0 + b2_0, (A h) w2_1 + b2_1])

Distribution (graph/data parallel): nodes are packed into 456 blocks of 128
slots (degree-balanced bin packing), 57 blocks per core.  Propagation for a
dst block accumulates S_j^T @ V_j over edge chunks j of 128 edges in PSUM,
where V_j is a dma_gather of the bf16 source-row table and S_j is a PURE 0/1
one-hot (edge -> dst slot) PRECOMPUTED ON HOST and streamed from HBM as bf16
(on-HW DVE tensor_scalar one-hot builds cost ~1.7us each; streaming is ~free).
The two D^-1/2 factors: gather tables are pre-scaled by dinv[src], and
dinv[dst] is applied once per block at PSUM eviction (it factors out of the
edge sum).  Reference self-loops are NOT in the edge list; they are the
identity matmul of the block's own (locally available) table rows.

dma_gather descriptor generation is the critical resource (~7ns/descriptor,
measured); calls are spread round-robin over all 4 SWDGE queues and use
exact per-(block,half) chunk counts padded to the max over cores at the same
sorted block position (so the SPMD program is identical on every core).

Cross-core: tables are replicated; x1 and z1 = dinv*(h @ w2_1) shards are
AllGather'd between propagations ((A h) w2_1 == A (h w2_1), so only 40 cols
propagate in conv2).  dma_gather needs int16 indices, so tables are split in
two 29184-row halves aligned with the node-id split.
"""
import sys

sys.path.insert(0, "/opt/trn_rl_repo")

import numpy as np

import concourse.bass as bass  # noqa: F401
import concourse.bacc as bacc
import concourse.tile as tile
from concourse import mybir
from concourse.bass_utils import run_bass_kernel_spmd

import ml_dtypes

BF16 = ml_dtypes.bfloat16

# ---- problem constants (hardcoded; kernel.py must be self-contained) ----
N = 50000
FIN = 128
H = 128
CO = 40
NCORES = 8
P = 128
NB = 456               # node blocks total
BPC = NB // NCORES     # 57 blocks per core
S = NB * P             # 58368 slots
NSH = BPC * P          # 7296 slots per core
HALF = S // 2          # 29184  (int16-addressable table half)
NHALF = N // 2         # node-id split matching the slot-half split
NQ = 4                 # SWDGE queues

LAST_RESULT = None     # BassKernelResults of the most recent run (for test.py)

_COMPILED = {}


# --------------------------------------------------------------------------
# host-side preprocessing
# --------------------------------------------------------------------------
def _pack_nodes(a, b):
    """Assign each node a slot: nodes [0,NHALF) -> blocks [0,NB/2), rest ->
    blocks [NB/2,NB).  Greedy 2D bin packing (descending total degree,
    minimize max(lo_load, hi_load)) balances both src-half in-edge counts."""
    node2slot = np.empty(N, np.int64)
    for hstart, hend, b0 in ((0, NHALF, 0), (NHALF, N, NB // 2)):
        nbins = NB // 2
        nodes = np.arange(hstart, hend)
        nodes = nodes[np.argsort(-(a[nodes] + b[nodes]), kind="stable")]
        lo_load = np.zeros(nbins, np.int64)
        hi_load = np.zeros(nbins, np.int64)
        cnt = np.zeros(nbins, np.int64)
        av = a[nodes]
        bv = b[nodes]
        for i in range(nodes.shape[0]):
            score = np.maximum(lo_load + av[i], hi_load + bv[i])
            score[cnt >= P] = 1 << 60
            blk = int(np.argmin(score))
            node2slot[nodes[i]] = (b0 + blk) * P + cnt[blk]
            cnt[blk] += 1
            lo_load[blk] += av[i]
            hi_load[blk] += bv[i]
    return node2slot


def _wrap_idx(flat):
    """[n*128] int16 -> [128, n*8] (16-partition wrap, replicated 8x)."""
    n = flat.shape[0] // 128
    arr = flat.reshape(n * 8, 16).T.copy()
    return np.tile(arr, (8, 1))


def _preprocess(x, edge_index, w1_0, b1_0, w1_1, b1_1, w1_2, b1_2,
                w2_0, b2_0, w2_1, b2_1):
    src = edge_index[0].astype(np.int64)
    dst = edge_index[1].astype(np.int64)
    E = src.shape[0]

    deg = (np.bincount(dst, minlength=N) + 1).astype(np.float32)  # +self loop
    dinv = (1.0 / np.sqrt(deg)).astype(np.float32)

    islo_n = src < NHALF
    a = np.bincount(dst[islo_n], minlength=N)
    b = np.bincount(dst[~islo_n], minlength=N)
    node2slot = _pack_nodes(a, b)

    # per-(old block, src half) edge counts, to sort/deal blocks to cores
    blk_old = node2slot[dst] >> 7
    grp_old = blk_old * 2 + (~islo_n).astype(np.int64)
    cnts = np.bincount(grp_old, minlength=2 * NB).reshape(NB, 2)
    tot = cnts.sum(axis=1)
    # block -> position: snake-deal blocks (sorted by total edges desc)
    # within each half so every core gets a similar block-size profile.
    order_bs = np.empty(NB, np.int64)  # new position -> old block id
    blk_perm = np.empty(NB, np.int64)  # old block id -> new position
    for half, coff in ((0, 0), (1, 4)):
        ids = np.arange(half * (NB // 2), (half + 1) * (NB // 2))
        ids = ids[np.argsort(-tot[ids], kind="stable")]
        percore = [[] for _ in range(4)]
        for i, bid in enumerate(ids):
            c = i % 4 if (i // 4) % 2 == 0 else 3 - i % 4
            percore[c].append(bid)
        for c in range(4):
            for j, bid in enumerate(percore[c]):
                pos = (coff + c) * BPC + j
                order_bs[pos] = bid
                blk_perm[bid] = pos
    slot_perm = (blk_perm[:, None] * P + np.arange(P)[None, :]).reshape(-1)
    node2slot = slot_perm[node2slot]
    cnts = cnts[order_bs]              # [position, half] edge counts

    dslot = node2slot[dst]
    nblk = dslot >> 7                  # block position (0..NB-1)
    d_in_blk = dslot & 127
    sslot = node2slot[src]
    islo = sslot < HALF                # == islo_n (halves preserved)
    sidx = np.where(islo, sslot, sslot - HALF).astype(np.int16)

    # chunk counts per (within-core position, half): max across cores
    ch = np.ceil(cnts / P).astype(np.int64).reshape(NCORES, BPC, 2)
    K_lo = np.maximum(ch[:, :, 0].max(axis=0), 1)  # [BPC]
    K_hi = np.maximum(ch[:, :, 1].max(axis=0), 1)
    Ktot = K_lo + K_hi
    TOTCH = int(Ktot.sum())            # chunks per core

    # flatten edges into the padded chunk layout
    grp = nblk * 2 + (~islo).astype(np.int64)
    order = np.argsort(grp, kind="stable")
    gs = np.bincount(grp, minlength=2 * NB)
    starts = np.concatenate([[0], np.cumsum(gs)[:-1]])
    epos = np.arange(E) - starts[grp[order]]    # position within group

    base_lo = np.concatenate([[0], np.cumsum(Ktot)[:-1]])  # [BPC] chunk base
    base_hi = base_lo + K_lo
    posc = np.arange(NB) % BPC
    gbase = np.stack([base_lo[posc], base_hi[posc]], axis=1)  # [NB, 2]
    eslot = gbase[nblk[order], (~islo[order]).astype(np.int64)] * P + epos
    core_e = nblk[order] // BPC

    sidx_pad = np.zeros((NCORES, TOTCH * P), np.int16)
    sidx_pad[core_e, eslot] = sidx[order]
    oh = np.zeros((NCORES, TOTCH * P, P), np.float32)
    oh[core_e, eslot, d_in_blk[order]] = 1.0

    dinv_slot = np.zeros(S, np.float32)
    dinv_slot[node2slot] = dinv
    x_slot = np.zeros((S, FIN), np.float32)
    x_slot[node2slot] = x
    u0 = (x_slot * dinv_slot[:, None]).astype(BF16)

    brow = np.tile(np.concatenate([b2_0, b2_1])[None, :], (P, 1)).astype(np.float32)
    w1s = np.concatenate([w1_0, w1_1, w1_2], axis=1).astype(BF16)
    b1m = np.stack([b1_0, b1_1, b1_2], axis=1).astype(np.float32)
    ident = np.eye(P, dtype=BF16)

    in_maps = []
    for c in range(NCORES):
        rows = slice(c * NSH, (c + 1) * NSH)
        dm = dinv_slot[rows].reshape(BPC, P).T.copy()
        # device one-hot layout: [128 edge-row, TOTCH*128 (chunk, dst)]
        ohc = np.ascontiguousarray(
            oh[c].reshape(TOTCH, P, P).transpose(1, 0, 2)
            .reshape(P, TOTCH * P)).astype(BF16)
        in_maps.append(dict(
            u0=u0,
            u0own=u0[rows],
            xT=np.ascontiguousarray(x_slot[rows].T).astype(BF16),
            idx=_wrap_idx(sidx_pad[c]),
            oh=ohc,
            dinvc=dm,
            dinv2c=(dm * dm),
            ident=ident,
            w1s=w1s,
            b1m=b1m,
            w2a=np.asarray(w2_0, np.float32).astype(BF16),
            w2b=np.asarray(w2_1, np.float32).astype(BF16),
            brow=brow,
        ))
    return in_maps, node2slot, tuple(int(v) for v in K_lo), \
        tuple(int(v) for v in K_hi)


# --------------------------------------------------------------------------
# device program
# --------------------------------------------------------------------------
def _build(nc, K_lo, K_hi, stage="full"):
    dt = mybir.dt
    f32 = dt.float32
    bf16 = dt.bfloat16
    Ktot = [a + b for a, b in zip(K_lo, K_hi)]
    TOTCH = sum(Ktot)
    KLO_M, KHI_M, KT_M = max(K_lo), max(K_hi), max(Ktot)
    cbase = [0]
    for k in Ktot:
        cbase.append(cbase[-1] + k)

    u0 = nc.dram_tensor("u0", [S, FIN], bf16, kind="ExternalInput").ap()
    u0own = nc.dram_tensor("u0own", [NSH, FIN], bf16, kind="ExternalInput").ap()
    xT = nc.dram_tensor("xT", [P, NSH], bf16, kind="ExternalInput").ap()
    idx = nc.dram_tensor("idx", [P, TOTCH * 8], dt.int16, kind="ExternalInput").ap()
    ohd = nc.dram_tensor("oh", [P, TOTCH * P], bf16, kind="ExternalInput").ap()
    dinvc = nc.dram_tensor("dinvc", [P, BPC], f32, kind="ExternalInput").ap()
    dinv2c = nc.dram_tensor("dinv2c", [P, BPC], f32, kind="ExternalInput").ap()
    identd = nc.dram_tensor("ident", [P, P], bf16, kind="ExternalInput").ap()
    w1s = nc.dram_tensor("w1s", [P, 3 * H], bf16, kind="ExternalInput").ap()
    b1m = nc.dram_tensor("b1m", [P, 3], f32, kind="ExternalInput").ap()
    w2a = nc.dram_tensor("w2a", [3 * H, CO], bf16, kind="ExternalInput").ap()
    w2b = nc.dram_tensor("w2b", [3 * H, CO], bf16, kind="ExternalInput").ap()
    brow = nc.dram_tensor("brow", [P, 2 * CO], f32, kind="ExternalInput").ap()
    out = nc.dram_tensor("out", [NSH, 2 * CO], f32, kind="ExternalOutput").ap()

    rg = [list(range(NCORES))]

    with tile.TileContext(nc) as tc:
        with (
            tc.tile_pool(name="res", bufs=1) as res,
            tc.tile_pool(name="dram", bufs=1, space="DRAM") as dram,
        ):
            def load(name, src_ap, shape, dtype=f32):
                t = res.tile(shape, dtype, tag=name, name=name)
                nc.sync.dma_start(out=t[:], in_=src_ap)
                return t

            idx_t = load("idx", idx[:], [P, TOTCH * 8], dt.int16)
            dinvc_t = load("dinvc", dinvc[:], [P, BPC])
            dinv2c_t = load("dinv2c", dinv2c[:], [P, BPC])
            ident_t = load("ident", identd[:], [P, P], bf16)
            w1_t = load("w1s", w1s[:], [P, 3 * H], bf16)
            b1_t = load("b1m", b1m[:], [P, 3])
            brow_t = load("brow", brow[:], [P, 2 * CO])
            w2a_t = [load(f"w2a{i}", w2a[i * H:(i + 1) * H, :], [P, CO], bf16)
                     for i in range(3)]
            w2b_t = [load(f"w2b{i}", w2b[i * H:(i + 1) * H, :], [P, CO], bf16)
                     for i in range(3)]

            x1T = res.tile([P, NSH], bf16, tag="x1T")
            x2T = res.tile([P, NSH], bf16, tag="x2T")
            hT = [res.tile([P, NSH], bf16, tag=f"hT{i}", name=f"hT{i}")
                  for i in range(3)]
            out80 = res.tile([P, BPC * 2 * CO], f32, tag="out80")

            u1b = dram.tile([NSH, FIN], bf16, tag="u1b")
            u1f = dram.tile([S, FIN], bf16, tag="u1f", addr_space="Shared")
            uzb = dram.tile([NSH, P], bf16, tag="uzb")
            uzf = dram.tile([S, P], bf16, tag="uzf", addr_space="Shared")

            qn = [0]  # SWDGE queue round-robin

            def gather_half(pw, tag, tbl, b, Ks, base_off, KM):
                Kp = Ks[b]
                v = pw.tile([P, KM, FIN], bf16, tag=tag, name=tag)
                o = base_off[b]
                nc.gpsimd.dma_gather(
                    v[:, 0:Kp, :], tbl, idx_t[:, o * 8:(o + Kp) * 8],
                    num_idxs=Kp * P, num_idxs_reg=Kp * P, elem_size=FIN,
                    queue_num=qn[0])
                qn[0] = (qn[0] + 1) % NQ
                return v

            base_lo = [cbase[b] for b in range(BPC)]
            base_hi = [cbase[b] + K_lo[b] for b in range(BPC)]

            def prop(tbl, own, width, evict, pools):
                pw, pp, ohp, sp = pools
                for b in range(BPC):
                    vlo = gather_half(pw, "vlo", tbl[0:HALF, :], b, K_lo,
                                      base_lo, KLO_M)
                    vhi = gather_half(pw, "vhi", tbl[HALF:S, :], b, K_hi,
                                      base_hi, KHI_M)
                    oht = ohp.tile([P, KT_M * P], bf16, tag="oht")
                    nc.sync.dma_start(
                        out=oht[:, 0:Ktot[b] * P],
                        in_=ohd[:, cbase[b] * P:cbase[b + 1] * P])
                    sblk = sp.tile([P, FIN], bf16, tag="sblk")
                    nc.sync.dma_start(out=sblk[:],
                                      in_=own[b * P:(b + 1) * P, :])
                    ps = pp.tile([P, width], f32, tag="agg")
                    nc.tensor.matmul(out=ps[:], lhsT=ident_t[:],
                                     rhs=sblk[:, 0:width],
                                     start=True, stop=False)
                    for j in range(Ktot[b]):
                        srcv = (vlo[:, j, 0:width] if j < K_lo[b]
                                else vhi[:, j - K_lo[b], 0:width])
                        nc.tensor.matmul(
                            out=ps[:],
                            lhsT=oht[:, j * P:(j + 1) * P],
                            rhs=srcv,
                            start=False, stop=(j == Ktot[b] - 1))
                    evict(b, ps)

            # ================= P1: x1 = A x =================
            with (
                tc.tile_pool(name="p1w", bufs=6) as pw,
                tc.tile_pool(name="p1p", bufs=4, space="PSUM") as pp,
                tc.tile_pool(name="p1o", bufs=3) as ohp,
                tc.tile_pool(name="p1s", bufs=2) as sp,
                tc.tile_pool(name="p1e", bufs=3) as evp,
                tc.tile_pool(name="p1t", bufs=2, space="PSUM") as tpp,
            ):
                def evict1(b, ps):
                    x1t = evp.tile([P, P], bf16, tag="x1t")
                    nc.scalar.mul(x1t[:], ps[:], dinvc_t[:, b:b + 1])
                    u1t = evp.tile([P, P], bf16, tag="u1t")
                    nc.scalar.mul(u1t[:], ps[:], dinv2c_t[:, b:b + 1])
                    nc.sync.dma_start(out=u1b[b * P:(b + 1) * P, :], in_=u1t[:])
                    trp = tpp.tile([P, P], bf16, tag="trp")
                    nc.tensor.transpose(out=trp[:], in_=x1t[:],
                                        identity=ident_t[:])
                    nc.vector.tensor_copy(out=x1T[:, b * P:(b + 1) * P],
                                          in_=trp[:])

                prop(u0, u0own, FIN, evict1, (pw, pp, ohp, sp))

            nc.gpsimd.collective_compute(
                "AllGather", mybir.AluOpType.bypass, replica_groups=rg,
                ins=[u1b.opt()], outs=[u1f.opt()])

            # hT[0] = relu(w1_0^T x^T + b1_0) only needs x -- run it in the
            # shadow of the first AllGather.
            with (
                tc.tile_pool(name="d0x", bufs=3) as xsp0,
                tc.tile_pool(name="d0p", bufs=3, space="PSUM") as hpp0,
            ):
                for f0 in range(0, NSH, 512):
                    w = min(512, NSH - f0)
                    xt = xsp0.tile([P, 512], bf16, tag="xs0")
                    nc.sync.dma_start(out=xt[:, 0:w], in_=xT[:, f0:f0 + w])
                    ph = hpp0.tile([P, 512], f32, tag="hps0")
                    nc.tensor.matmul(out=ph[:, 0:w], lhsT=w1_t[:, 0:H],
                                     rhs=xt[:, 0:w], start=True, stop=True)
                    nc.scalar.activation(
                        out=hT[0][:, f0:f0 + w], in_=ph[:, 0:w],
                        func=mybir.ActivationFunctionType.Relu,
                        bias=b1_t[:, 0:1], scale=1.0)

            if stage == "p1":
                dbg = nc.dram_tensor("dbg", [S, FIN], f32,
                                     kind="ExternalOutput").ap()
                with tc.tile_pool(name="dbgp", bufs=2) as dp:
                    for r0 in range(0, S, P):
                        t = dp.tile([P, FIN], f32, tag="dbgt")
                        nc.vector.tensor_copy(out=t[:], in_=u1f[r0:r0 + P, :])
                        nc.sync.dma_start(out=dbg[r0:r0 + P, :], in_=t[:])
                return

            # ================= P2: x2 = A x1 =================
            with (
                tc.tile_pool(name="p2w", bufs=6) as pw,
                tc.tile_pool(name="p2p", bufs=4, space="PSUM") as pp,
                tc.tile_pool(name="p2o", bufs=3) as ohp,
                tc.tile_pool(name="p2s", bufs=2) as sp,
                tc.tile_pool(name="p2e", bufs=3) as evp,
                tc.tile_pool(name="p2t", bufs=2, space="PSUM") as tpp,
            ):
                def evict2(b, ps):
                    x2t = evp.tile([P, P], bf16, tag="x2t")
                    nc.scalar.mul(x2t[:], ps[:], dinvc_t[:, b:b + 1])
                    trp = tpp.tile([P, P], bf16, tag="trp2")
                    nc.tensor.transpose(out=trp[:], in_=x2t[:],
                                        identity=ident_t[:])
                    nc.vector.tensor_copy(out=x2T[:, b * P:(b + 1) * P],
                                          in_=trp[:])

                prop(u1f, u1b, FIN, evict2, (pw, pp, ohp, sp))

            if stage == "p2":
                dbg = nc.dram_tensor("dbg", [2 * P, NSH], f32,
                                     kind="ExternalOutput").ap()
                t = res.tile([P, NSH], f32, tag="dbgt")
                nc.vector.tensor_copy(out=t[:], in_=x1T[:])
                nc.sync.dma_start(out=dbg[0:P, :], in_=t[:])
                nc.vector.tensor_copy(out=t[:], in_=x2T[:])
                nc.sync.dma_start(out=dbg[P:2 * P, :], in_=t[:])
                return

            # ================= dense: hT = relu(w1^T [x|x1|x2]^T + b1) ======
            with (
                tc.tile_pool(name="dxs", bufs=3) as xsp,
                tc.tile_pool(name="dps", bufs=3, space="PSUM") as hpp,
            ):
                for f0 in range(0, NSH, 512):
                    w = min(512, NSH - f0)
                    srcs = (None, x1T[:, f0:f0 + w], x2T[:, f0:f0 + w])
                    for i in (1, 2):
                        ph = hpp.tile([P, 512], f32, tag="hps")
                        nc.tensor.matmul(out=ph[:, 0:w],
                                         lhsT=w1_t[:, i * H:(i + 1) * H],
                                         rhs=srcs[i], start=True, stop=True)
                        nc.scalar.activation(
                            out=hT[i][:, f0:f0 + w], in_=ph[:, 0:w],
                            func=mybir.ActivationFunctionType.Relu,
                            bias=b1_t[:, i:i + 1], scale=1.0)

            if stage == "dense":
                dbg = nc.dram_tensor("dbg", [3 * P, NSH], f32,
                                     kind="ExternalOutput").ap()
                t = res.tile([P, NSH], f32, tag="dbgt")
                for i in range(3):
                    nc.vector.tensor_copy(out=t[:], in_=hT[i][:])
                    nc.sync.dma_start(out=dbg[i * P:(i + 1) * P, :], in_=t[:])
                return

            # ================= z1 = dinv*(h w2_1) (-> uz), out1 = h w2_0 ====
            with (
                tc.tile_pool(name="eps", bufs=4, space="PSUM") as zpp,
                tc.tile_pool(name="eev", bufs=3) as evp,
            ):
                for b in range(BPC):
                    pz = zpp.tile([P, CO], f32, tag="pz")
                    for i in range(3):
                        nc.tensor.matmul(out=pz[:],
                                         lhsT=hT[i][:, b * P:(b + 1) * P],
                                         rhs=w2b_t[i][:], start=(i == 0),
                                         stop=(i == 2))
                    uzt = evp.tile([P, P], bf16, tag="uzt")
                    nc.vector.memset(uzt[:, CO:P], 0)
                    nc.scalar.mul(uzt[:, 0:CO], pz[:], dinvc_t[:, b:b + 1])
                    nc.sync.dma_start(out=uzb[b * P:(b + 1) * P, :], in_=uzt[:])

                nc.gpsimd.collective_compute(
                    "AllGather", mybir.AluOpType.bypass, replica_groups=rg,
                    ins=[uzb.opt()], outs=[uzf.opt()])

                # out1 = h w2_0 does not depend on the AllGather -- run it in
                # the collective's shadow.
                for b in range(BPC):
                    po = zpp.tile([P, CO], f32, tag="po")
                    for i in range(3):
                        nc.tensor.matmul(out=po[:],
                                         lhsT=hT[i][:, b * P:(b + 1) * P],
                                         rhs=w2a_t[i][:], start=(i == 0),
                                         stop=(i == 2))
                    nc.vector.tensor_copy(
                        out=out80[:, b * 2 * CO:b * 2 * CO + CO], in_=po[:])

            # ========== P3: out2 = dinv * A' z1, fused bias+softmax-head ====
            # Per-block softmax prologue (bias, max, shift, exp+accum) rides
            # inside the gather-bound P3 loop; the Ln over all 57 row-sums is
            # ONE activation at the end (the ACT engine reloads its function
            # table on Exp<->Ln switches, 1.3us each -- batching avoids 2*57
            # reloads).
            sh_all = res.tile([P, BPC * 2 * CO], f32, tag="sh_all")
            se_all = res.tile([P, BPC], f32, tag="se_all")
            LAG = 20  # the ACT exp trails by LAG blocks so its DVE-chain
            #           inputs are long since ready and it never head-of-line
            #           blocks the PSUM-releasing mul of the current block.
            with (
                tc.tile_pool(name="p3w", bufs=6) as pw,
                tc.tile_pool(name="p3p", bufs=4, space="PSUM") as pp,
                tc.tile_pool(name="p3o", bufs=3) as ohp,
                tc.tile_pool(name="p3s", bufs=2) as sp,
                tc.tile_pool(name="p3f", bufs=4) as fp,
            ):
                def smax_dve(b):
                    t1 = fp.tile([P, 2 * CO], f32, tag="f1")
                    nc.vector.tensor_tensor(
                        out=t1[:], in0=out80[:, b * 2 * CO:(b + 1) * 2 * CO],
                        in1=brow_t[:], op=mybir.AluOpType.add)
                    mx = fp.tile([P, 1], f32, tag="mx")
                    nc.vector.reduce_max(out=mx[:], in_=t1[:],
                                         axis=mybir.AxisListType.X)
                    sh = sh_all[:, b * 2 * CO:(b + 1) * 2 * CO]
                    nc.vector.tensor_scalar(out=sh, in0=t1[:], scalar1=mx[:],
                                            scalar2=None,
                                            op0=mybir.AluOpType.subtract)

                def smax_exp(b):
                    ex = fp.tile([P, 2 * CO], f32, tag="ex")
                    nc.scalar.activation(
                        out=ex[:], in_=sh_all[:, b * 2 * CO:(b + 1) * 2 * CO],
                        func=mybir.ActivationFunctionType.Exp,
                        accum_out=se_all[:, b:b + 1])

                def evict3(b, ps):
                    nc.scalar.mul(
                        out80[:, b * 2 * CO + CO:(b + 1) * 2 * CO], ps[:],
                        dinvc_t[:, b:b + 1])
                    smax_dve(b)
                    if b >= LAG:
                        smax_exp(b - LAG)

                prop(uzf, uzb, CO, evict3, (pw, pp, ohp, sp))
                for b in range(BPC - LAG, BPC):
                    smax_exp(b)

            # ================= lse + final subtract + store ==================
            with tc.tile_pool(name="fin", bufs=3) as fp:
                lse = res.tile([P, BPC], f32, tag="lse")
                nc.scalar.activation(out=lse[:], in_=se_all[:],
                                     func=mybir.ActivationFunctionType.Ln)
                for b in range(BPC):
                    r = fp.tile([P, 2 * CO], f32, tag="r")
                    nc.vector.tensor_scalar(
                        out=r[:], in0=sh_all[:, b * 2 * CO:(b + 1) * 2 * CO],
                        scalar1=lse[:, b:b + 1], scalar2=None,
                        op0=mybir.AluOpType.subtract)
                    nc.sync.dma_start(out=out[b * P:(b + 1) * P, :], in_=r[:])


def _get_compiled(K_lo, K_hi, stage="full"):
    key = (K_lo, K_hi, stage)
    if key not in _COMPILED:
        nc = bacc.Bacc("TRN2", target_bir_lowering=False, debug=False,
                       num_devices=NCORES, num_swdge_queues=NQ)
        _build(nc, K_lo, K_hi, stage=stage)
        nc.compile()
        _COMPILED[key] = nc
    return _COMPILED[key]


def kernel(**inputs):
    global LAST_RESULT
    args = {k: np.asarray(v) for k, v in inputs.items()}
    in_maps, node2slot, K_lo, K_hi = _preprocess(
        args["x"].astype(np.float32), args["edge_index"],
        args["w1_0"].astype(np.float32), args["b1_0"].astype(np.float32),
        args["w1_1"].astype(np.float32), args["b1_1"].astype(np.float32),
        args["w1_2"].astype(np.float32), args["b1_2"].astype(np.float32),
        args["w2_0"].astype(np.float32), args["b2_0"].astype(np.float32),
        args["w2_1"].astype(np.float32), args["b2_1"].astype(np.float32),
    )
    nc = _get_compiled(K_lo, K_hi)
    res = run_bass_kernel_spmd(nc, in_maps, list(range(NCORES)))
    LAST_RESULT = res
    out_slot = np.concatenate([res.results[c]["out"] for c in range(NCORES)],
                              axis=0)
    return out_slot[node2slot].astype(np.float32)
